# revision 1
# baseline (speedup 1.0000x reference)
"""Multi-head attention forward, tensor-parallel over heads across 8 TRN2 cores.

Problem: B=4, S=2048, D=1024, H=16, DK=64.
  qkv = x @ Wqkv.T + bqkv ; per-head scaled-dot-product attention (no mask);
  out = attn_out @ Wout.T + bout

Sharding: 2 heads per core. Each core computes the QKV projection for its 2
heads (full sequence) and their attention; an AllToAll then redistributes
head-features to token-slices so each core runs the output projection for
1/8 of the tokens.

Per core:
  - All matmuls in bf16 (fp32 PSUM). x is cast f32->bf16 during its SWDGE
    DMA loads; x^T tiles produced by PE transposes.
  - Q^T/K^T feature-major [128 feat(2 heads x 64), 8192 tok] resident SBUF.
  - V transposed to token-major with a fused ones-column (V') so the P@V
    matmul also produces softmax row-sums for free (PSUM row 64).
  - scores computed transposed: S^T[tk, tq] = K^T.T @ Q^T. The two heads'
    matmuls are issued back-to-back as 64-row PE tiles into one shared PSUM
    slab (different banks), so they run concurrently in the array and a
    single ACT exp op covers both heads (1/sqrt(dk) folded into the scale).
  - normalize: O^T [65, tq] -> PE transpose to token-major, per-partition
    reciprocal of the row-sum column, tensor_scalar_mul.
"""
import os
import sys

import numpy as np

sys.path.insert(0, "/opt/trn_rl_repo")

import concourse.bass as bass
import concourse.mybir as mybir
import concourse.tile as tile
from concourse import bacc
from concourse.bass_utils import run_bass_kernel_spmd
from concourse.masks import make_identity

F32 = mybir.dt.float32
BF16 = mybir.dt.bfloat16

N_CORES = 8
B, S, D, H = 4, 2048, 1024, 16
DK = D // H
T = B * S  # 8192 flattened tokens
HPC = H // N_CORES  # heads per core = 2
FPC = HPC * DK  # features per core = 128
TPC = T // N_CORES  # tokens per core for out-proj = 1024

QC = 256  # q-chunk (moving dim of scores / PV matmuls)
STT = 512  # phase-1 token super-tile
TKC = 128  # k-token chunk (partition dim of S^T tiles)
N_TKC = S // TKC  # 16
EXP_GRP = 2  # tk-chunks per dual-head ACT exp op (free = 2*EXP_GRP*QC)

AluOp = mybir.AluOpType
ActFn = mybir.ActivationFunctionType

_CACHE = {}


def _build():
    nc = bacc.Bacc("TRN2", target_bir_lowering=False, debug=False,
                   num_devices=N_CORES)

    xf = nc.dram_tensor("xf", [T, D], F32, kind="ExternalInput")
    wqkvt = nc.dram_tensor("wqkvt", [D, 3 * FPC], F32, kind="ExternalInput")
    bqkv3 = nc.dram_tensor("bqkv3", [FPC, 3], F32, kind="ExternalInput")
    woutt = nc.dram_tensor("woutt", [D, D], F32, kind="ExternalInput")
    boutr = nc.dram_tensor("boutr", [1, D], F32, kind="ExternalInput")
    y = nc.dram_tensor("y", [TPC, D], F32, kind="ExternalOutput")

    with tile.TileContext(nc) as tc:
        with (
            tc.tile_pool(name="dram", bufs=1, space="DRAM") as dram,
            tc.tile_pool(name="consts", bufs=1) as consts,
        ):
            # token-major bf16 attention output, [dest, token, feature]
            send = dram.tile([N_CORES, TPC, FPC], BF16)
            recv = dram.tile([N_CORES, TPC, FPC], BF16)

            identity = consts.tile([128, 128], BF16)
            make_identity(nc, identity)
            identity_f32 = consts.tile([128, 128], F32)
            make_identity(nc, identity_f32)

            with tc.tile_pool(name="ph12", bufs=1) as ph12:
                w_sb = ph12.tile([128, 8, 3 * FPC], BF16)  # [d_chunk, dc, f]
                nc.gpsimd.dma_start(
                    out=w_sb,
                    in_=wqkvt.ap().rearrange("(dc p) f -> p dc f", p=128))
                b_sb = ph12.tile([FPC, 3], F32)
                nc.sync.dma_start(out=b_sb, in_=bqkv3[:, :])

                # resident QKV^T slab: [128 feat, {q,k,v}, 8192 tok]
                qkvt = ph12.tile([128, 3, T], BF16)
                # V' token-major slab: [128 tk, b, tkc, h, 65] (col64=ones)
                vp = ph12.tile([128, B, N_TKC, HPC, 66], BF16)
                nc.vector.memset(vp[:, :, :, :, 64:65], 1.0)

                self_attention(nc, tc, xf, send, identity, identity_f32,
                               w_sb, b_sb, qkvt, vp)

            out_projection(nc, tc, woutt, boutr, y, send, recv, identity)

    nc.compile()
    return nc


def self_attention(nc, tc, xf, send, identity, identity_f32,
                   w_sb, b_sb, qkvt, vp):
    # ---------------- Phase 1: QKV projection ----------------
    with (
        tc.tile_pool(name="xin", bufs=3) as xin_pool,
        tc.tile_pool(name="xt", bufs=2) as xt_pool,
        tc.tile_pool(name="tr_ps", bufs=4, space="PSUM") as tr_ps,
        tc.tile_pool(name="qkv_ps", bufs=3, space="PSUM") as qkv_ps,
    ):
        for st in range(T // STT):
            t0 = st * STT
            xs = []
            for r in range(STT // 128):
                xr = xin_pool.tile([128, D], BF16, tag=f"x{r}",
                                   name=f"x{r}_{st}")
                # SWDGE casts f32 -> bf16 during the load
                nc.gpsimd.dma_start(
                    out=xr, in_=xf[t0 + r * 128:t0 + (r + 1) * 128, :])
                xs.append(xr)
            xt = xt_pool.tile([128, 8, STT], BF16)
            for dc in range(8):
                for r, xsrc in enumerate(xs):
                    pst = tr_ps.tile([128, 128], BF16)
                    nc.tensor.transpose(
                        pst, xsrc[:, dc * 128:(dc + 1) * 128], identity)
                    # alternate evacuation between DVE and ACT
                    if (dc + r) % 2 == 0:
                        nc.vector.tensor_copy(
                            xt[:, dc, r * 128:(r + 1) * 128], pst)
                    else:
                        nc.scalar.copy(
                            xt[:, dc, r * 128:(r + 1) * 128], pst)
            for fc in range(3):
                ps = qkv_ps.tile([128, STT], F32)
                for dc in range(8):
                    nc.tensor.matmul(
                        ps,
                        w_sb[:, dc, fc * FPC:(fc + 1) * FPC],
                        xt[:, dc, :],
                        start=(dc == 0), stop=(dc == 7))
                nc.vector.tensor_scalar_add(
                    qkvt[:, fc, t0:t0 + STT], ps, b_sb[:, fc:fc + 1])

            # V' for the k-chunks this supertile just produced
            b = t0 // S
            kc0 = (t0 % S) // TKC
            for kc in range(kc0, kc0 + STT // TKC):
                tk0 = b * S + kc * TKC
                pst = tr_ps.tile([128, 128], BF16)
                nc.tensor.transpose(pst, qkvt[:, 2, tk0:tk0 + TKC], identity)
                for h in range(HPC):
                    nc.scalar.copy(
                        vp[:, b, kc, h, 0:DK],
                        pst[:, h * DK:(h + 1) * DK])

    # ---------------- Phase 2: attention ----------------
    with (
        tc.tile_pool(name="p_slab", bufs=1) as p_pool,
        tc.tile_pool(name="s_ps", bufs=2, space="PSUM") as s_ps,
        tc.tile_pool(name="o_ps", bufs=2, space="PSUM") as o_ps,
        tc.tile_pool(name="otr_ps", bufs=2, space="PSUM") as otr_ps,
        tc.tile_pool(name="norm", bufs=6) as norm_pool,
        tc.tile_pool(name="stage", bufs=4) as stage_pool,
    ):
        def flush_normalize(nc, q0, o65s):
            # deferred: token-major transpose, 1/rowsum, scale, send
            stg = stage_pool.tile([128, QC // 128, HPC, DK], BF16,
                                  tag="stg", name=f"stg{q0}")
            for h in range(HPC):
                for r in range(QC // 128):
                    otr = otr_ps.tile([128, DK + 1], F32, tag="otr",
                                      name=f"otr{q0}_{h}_{r}")
                    nc.tensor.transpose(
                        otr, o65s[h][:, r * 128:(r + 1) * 128],
                        identity_f32[0:DK + 1, 0:DK + 1])
                    rcp = norm_pool.tile([128, 1], F32, tag="rcp",
                                         name=f"rcp{q0}_{h}_{r}")
                    nc.vector.reciprocal(rcp, otr[:, DK:DK + 1])
                    nc.vector.tensor_scalar_mul(
                        stg[:, r, h, :], otr[:, 0:DK], rcp)
            sl = q0 // TPC
            off = q0 % TPC
            for r in range(QC // 128):
                nc.sync.dma_start(
                    out=send[sl, off + r * 128:off + (r + 1) * 128, :],
                    in_=stg[:, r, :, :])

        pending = None  # (q0, [o65 per head]) awaiting normalize
        for b in range(B):
            for qi in range(S // QC):
                q0 = b * S + qi * QC
                # combined P^T slab for both heads: [p, h, tkc, tq] bf16
                pcomb = p_pool.tile([128, HPC, N_TKC, QC], BF16, tag="pc",
                                    name=f"pc{q0}")
                for g in range(N_TKC // EXP_GRP):
                    # dual-head score slab: [p, h, j, tq] f32 (2 banks)
                    sp = s_ps.tile([128, HPC, EXP_GRP, QC], F32, tag="sp",
                                   name=f"sp{q0}_{g}")
                    for j in range(EXP_GRP):
                        kc = g * EXP_GRP + j
                        tk0 = b * S + kc * TKC
                        for h in range(HPC):
                            kt = qkvt[h * DK:(h + 1) * DK, 1,
                                      tk0:tk0 + TKC]
                            qt = qkvt[h * DK:(h + 1) * DK, 0, q0:q0 + QC]
                            nc.tensor.matmul(
                                sp[:, h, j, :], kt, qt,
                                start=True, stop=True,
                                tile_position=(h * DK, 0))
                    nc.scalar.activation(
                        pcomb[:, :, g * EXP_GRP:(g + 1) * EXP_GRP, :],
                        sp, ActFn.Exp, scale=1.0 / 8.0)
                # previous iteration's normalize runs while exp proceeds
                if pending is not None:
                    flush_normalize(nc, *pending)
                    pending = None
                o65s = []
                for h in range(HPC):
                    op = o_ps.tile([128, QC], F32, tag="op",
                                   name=f"op{q0}_{h}")
                    for kc in range(N_TKC):
                        nc.tensor.matmul(
                            op[0:DK + 1, :],
                            vp[:, b, kc, h, 0:DK + 1],
                            pcomb[:, h, kc, :],
                            start=(kc == 0), stop=(kc == N_TKC - 1))
                    o65 = norm_pool.tile([DK + 1, QC], F32, tag="o65",
                                         name=f"o65_{q0}_{h}")
                    nc.vector.tensor_copy(o65, op[0:DK + 1, :])
                    o65s.append(o65)
                pending = (q0, o65s)
        flush_normalize(nc, *pending)


def out_projection(nc, tc, woutt, boutr, y, send, recv, identity):
    # ---------------- Phase 3: AllToAll + out projection ----------------
    with (
        tc.tile_pool(name="wout", bufs=1) as wout_pool,
        tc.tile_pool(name="oin", bufs=1) as oin_pool,
        tc.tile_pool(name="rt", bufs=1) as rt_pool,
        tc.tile_pool(name="tr3_ps", bufs=4, space="PSUM") as tr3_ps,
        tc.tile_pool(name="y_ps", bufs=2, space="PSUM") as y_ps,
        tc.tile_pool(name="yout", bufs=4) as yout_pool,
    ):
        wout_sb = wout_pool.tile([128, 8, D], BF16)  # [f_chunk, fc, e]
        nc.gpsimd.dma_start(
            out=wout_sb,
            in_=woutt.ap().rearrange("(fc p) e -> p fc e", p=128))
        bout_sb = wout_pool.tile([128, D], F32)
        bout_bcast = bass.AP(
            tensor=boutr.ap().tensor,
            offset=boutr.ap().offset,
            ap=[[0, 128], boutr.ap().ap[1]])
        nc.gpsimd.dma_start(out=bout_sb, in_=bout_bcast)

        nc.gpsimd.collective_compute(
            "AllToAll",
            AluOp.bypass,
            replica_groups=[list(range(N_CORES))],
            ins=[send.opt()],
            outs=[recv.opt()],
        )

        # prefetch ALL received tiles first so DMA latency never stalls
        # the PE transpose chain, then transpose back to feature-major
        o_sb = oin_pool.tile([128, 8, TPC], BF16)  # [f_in_chunk, fc, t]
        rts = {}
        for fg in range(8):
            for r in range(TPC // 128):
                rt = rt_pool.tile([128, FPC], BF16, tag=f"rt{fg}_{r}",
                                  name=f"rt{fg}_{r}")
                nc.sync.dma_start(
                    out=rt, in_=recv[fg, r * 128:(r + 1) * 128, :])
                rts[(fg, r)] = rt
        for fg in range(8):
            for r in range(TPC // 128):
                ptr = tr3_ps.tile([128, 128], BF16)
                nc.tensor.transpose(ptr, rts[(fg, r)], identity)
                if (fg + r) % 2 == 0:
                    nc.vector.tensor_copy(
                        o_sb[:, fg, r * 128:(r + 1) * 128], ptr)
                else:
                    nc.scalar.copy(
                        o_sb[:, fg, r * 128:(r + 1) * 128], ptr)

        for tt in range(TPC // 128):
            for ec in range(D // 512):
                ps = y_ps.tile([128, 512], F32)
                for fc in range(8):
                    nc.tensor.matmul(
                        ps,
                        o_sb[:, fc, tt * 128:(tt + 1) * 128],
                        wout_sb[:, fc, ec * 512:(ec + 1) * 512],
                        start=(fc == 0), stop=(fc == 7))
                yt = yout_pool.tile([128, 512], F32)
                nc.vector.tensor_add(
                    yt, ps, bout_sb[:, ec * 512:(ec + 1) * 512])
                nc.sync.dma_start(
                    out=y[tt * 128:(tt + 1) * 128, ec * 512:(ec + 1) * 512],
                    in_=yt)


def _get_nc():
    if "nc" not in _CACHE:
        _CACHE["nc"] = _build()
    return _CACHE["nc"]


def kernel(x, Wqkv, bqkv, Wout, bout):
    x = np.ascontiguousarray(np.asarray(x, dtype=np.float32))
    Wqkv = np.asarray(Wqkv, dtype=np.float32)
    bqkv = np.asarray(bqkv, dtype=np.float32)
    Wout = np.asarray(Wout, dtype=np.float32)
    bout = np.asarray(bout, dtype=np.float32)

    xf = x.reshape(T, D)
    woutt = np.ascontiguousarray(Wout.T)  # [f, e]
    boutr = bout.reshape(1, D)

    in_maps = []
    for c in range(N_CORES):
        f0 = c * FPC  # first feature row of this core's heads
        rows = np.concatenate([
            Wqkv[f0:f0 + FPC],                  # q rows
            Wqkv[D + f0:D + f0 + FPC],          # k rows
            Wqkv[2 * D + f0:2 * D + f0 + FPC],  # v rows
        ])  # [384, 1024]
        wqkvt = np.ascontiguousarray(rows.T)  # [1024, 384]
        bq = np.concatenate([
            bqkv[f0:f0 + FPC],
            bqkv[D + f0:D + f0 + FPC],
            bqkv[2 * D + f0:2 * D + f0 + FPC],
        ])  # [384]
        bqkv3 = np.ascontiguousarray(bq.reshape(3, FPC).T)  # [128, 3]
        in_maps.append({
            "xf": xf,
            "wqkvt": wqkvt,
            "bqkv3": bqkv3,
            "woutt": woutt,
            "boutr": boutr,
        })

    nc = _get_nc()
    trace = os.environ.get("MHA_TRACE") == "1"
    res = run_bass_kernel_spmd(
        nc, in_maps, core_ids=list(range(N_CORES)), trace=trace)
    if trace:
        _CACHE["last_result"] = res

    out = np.concatenate([res.results[c]["y"] for c in range(N_CORES)], axis=0)
    return out.reshape(B, S, D)



# revision 4
# speedup vs baseline: 1.0434x; 1.0434x over previous
"""Multi-head attention forward, tensor-parallel over heads across 8 TRN2 cores.

Problem: B=4, S=2048, D=1024, H=16, DK=64.
  qkv = x @ Wqkv.T + bqkv ; per-head scaled-dot-product attention (no mask);
  out = attn_out @ Wout.T + bout

Sharding: 2 heads per core. Each core computes the QKV projection for its 2
heads (full sequence) and their attention. Work is software-pipelined per
batch: QKV(b+1) overlaps attention(b); a per-batch AllToAll redistributes
head-features to token-slices and overlaps attention(b+1); the output
projection for received tokens overlaps subsequent batches.

Per core:
  - x is pre-transposed and pre-cast to bf16 on the host ([D, T] layout), so
    no PE transposes are spent on x^T; Q^T/K^T come out of the QKV matmul
    feature-major directly.
  - V transposed to token-major with a fused ones-column (V') so the P@V
    matmul also produces softmax row-sums for free (PSUM row 64).
  - scores computed transposed: S^T[tk, tq] = K^T.T @ Q^T with QC=512 moving.
    The two heads' matmuls are issued as 64-row PE tiles (tile_position) into
    one shared 2-bank PSUM slab; one ACT exp op covers both heads per
    k-chunk (1/sqrt(dk) folded into the activation scale).
  - normalize: O^T [65, tq] -> PE transpose to token-major, reciprocal of the
    row-sum column, tensor_scalar_mul, then PE transpose back to
    feature-major so receivers can matmul the out-projection directly.
  - Out-proj token ownership: dest core d owns tokens [b*2048 + d*256, +256)
    for every batch b, so each batch's attention output forms a complete
    8-way AllToAll; the host reassembles the interleaved result.
"""
import os
import sys

import numpy as np
import ml_dtypes

sys.path.insert(0, "/opt/trn_rl_repo")

import concourse.bass as bass
import concourse.mybir as mybir
import concourse.tile as tile
from concourse import bacc
from concourse.bass_utils import run_bass_kernel_spmd
from concourse.masks import make_identity

F32 = mybir.dt.float32
BF16 = mybir.dt.bfloat16
BF16NP = ml_dtypes.bfloat16

N_CORES = 8
B, S, D, H = 4, 2048, 1024, 16
DK = D // H
T = B * S  # 8192 flattened tokens
HPC = H // N_CORES  # heads per core = 2
FPC = HPC * DK  # features per core = 128
TPC = T // N_CORES  # tokens per core for out-proj = 1024
TPB = TPC // B  # out-proj tokens per core per batch = 256

QC = 512  # q-chunk (moving dim of scores / PV matmuls)
NQC = S // QC  # 4 q-chunks per batch
STT = 512  # QKV token super-tile
TKC = 128  # k-token chunk (partition dim of S^T tiles)
N_TKC = S // TKC  # 16

AluOp = mybir.AluOpType
ActFn = mybir.ActivationFunctionType

_CACHE = {}


def _build():
    nc = bacc.Bacc("TRN2", target_bir_lowering=False, debug=False,
                   num_devices=N_CORES)

    xtb = nc.dram_tensor("xtb", [D, T], BF16, kind="ExternalInput")
    wqkvt = nc.dram_tensor("wqkvt", [D, 3 * FPC], BF16, kind="ExternalInput")
    bqkv3 = nc.dram_tensor("bqkv3", [FPC, 3], F32, kind="ExternalInput")
    woutt = nc.dram_tensor("woutt", [D, D], BF16, kind="ExternalInput")
    boutr = nc.dram_tensor("boutr", [1, D], F32, kind="ExternalInput")
    y = nc.dram_tensor("y", [TPC, D], F32, kind="ExternalOutput")

    with tile.TileContext(nc) as tc:
        with (
            tc.tile_pool(name="dram", bufs=1, space="DRAM") as dram,
            tc.tile_pool(name="consts", bufs=1) as consts,
            tc.tile_pool(name="qkvt", bufs=2) as qkvt_pool,
            tc.tile_pool(name="vp", bufs=2) as vp_pool,
            tc.tile_pool(name="xt", bufs=2) as xt_pool,
            tc.tile_pool(name="pcomb", bufs=2) as pcomb_pool,
            tc.tile_pool(name="o65", bufs=2) as o65_pool,
            tc.tile_pool(name="stg", bufs=2) as stg_pool,
            tc.tile_pool(name="sstg", bufs=2) as sstg_pool,
            tc.tile_pool(name="osb", bufs=2) as osb_pool,
            tc.tile_pool(name="yt", bufs=2) as yt_pool,
            tc.tile_pool(name="rcp", bufs=4) as rcp_pool,
            tc.tile_pool(name="qkv_ps", bufs=1, space="PSUM") as qkv_ps,
            tc.tile_pool(name="s_ps", bufs=2, space="PSUM") as s_ps,
            tc.tile_pool(name="o_ps", bufs=1, space="PSUM") as o_ps,
            tc.tile_pool(name="tr_ps", bufs=1, space="PSUM") as tr_ps,
            tc.tile_pool(name="y_ps", bufs=1, space="PSUM") as y_ps,
        ):
            sends = [dram.tile([N_CORES, FPC, TPB], BF16, name=f"send{b}")
                     for b in range(B)]
            recvs = [dram.tile([N_CORES, FPC, TPB], BF16, name=f"recv{b}")
                     for b in range(B)]

            identity = consts.tile([128, 128], BF16)
            make_identity(nc, identity)
            identity_f32 = consts.tile([128, 128], F32)
            make_identity(nc, identity_f32)

            w_sb = consts.tile([128, 8, 3 * FPC], BF16)  # [d_chunk, dc, f]
            nc.sync.dma_start(
                out=w_sb,
                in_=wqkvt.ap().rearrange("(dc p) f -> p dc f", p=128))
            b_sb = consts.tile([FPC, 3], F32)
            nc.sync.dma_start(out=b_sb, in_=bqkv3[:, :])
            wout_sb = consts.tile([128, 8, D], BF16)  # [f_chunk, fc, e]
            nc.sync.dma_start(
                out=wout_sb,
                in_=woutt.ap().rearrange("(fc p) e -> p fc e", p=128))
            bout_sb = consts.tile([128, D], F32)
            bout_bcast = bass.AP(
                tensor=boutr.ap().tensor,
                offset=boutr.ap().offset,
                ap=[[0, 128], boutr.ap().ap[1]])
            nc.gpsimd.dma_start(out=bout_sb, in_=bout_bcast)

            xtb_r = xtb.ap().rearrange("(dc p) t -> p dc t", p=128)

            qkvts = {}
            vps = {}

            def emit_qkv_supertile(b, st):
                # QKV projection for batch b, tokens [st*512, (st+1)*512)
                if b not in qkvts:
                    qkvts[b] = qkvt_pool.tile([128, 3, S], BF16, tag="qkvt",
                                              name=f"qkvt{b}")
                    vps[b] = vp_pool.tile([128, N_TKC, HPC, 66], BF16,
                                          tag="vp", name=f"vp{b}")
                    nc.gpsimd.memset(vps[b][:, :, :, 64:65], 1.0)
                qkvt_b, vp_b = qkvts[b], vps[b]
                t0 = b * S + st * STT
                xt = xt_pool.tile([128, 8, STT], BF16, tag="xt",
                                  name=f"xt{b}_{st}")
                nc.sync.dma_start(out=xt, in_=xtb_r[:, :, t0:t0 + STT])
                for fc in range(3):
                    ps = qkv_ps.tile([128, STT], F32, tag="qps")
                    for dc in range(8):
                        nc.tensor.matmul(
                            ps,
                            w_sb[:, dc, fc * FPC:(fc + 1) * FPC],
                            xt[:, dc, :],
                            start=(dc == 0), stop=(dc == 7))
                    nc.vector.tensor_scalar_add(
                        qkvt_b[:, fc, st * STT:(st + 1) * STT], ps,
                        b_sb[:, fc:fc + 1])
                # V' token-major for the k-chunks this supertile produced
                for kc in range(st * (STT // TKC), (st + 1) * (STT // TKC)):
                    pst = tr_ps.tile([128, 128], BF16, tag="tr",
                                     name=f"vtr{b}_{kc}")
                    nc.tensor.transpose(
                        pst, qkvt_b[:, 2, kc * TKC:(kc + 1) * TKC], identity)
                    nc.vector.tensor_copy(
                        vp_b[:, kc, :, 0:DK],
                        pst.rearrange("p (h k) -> p h k", h=HPC))

            def emit_scores_exp(b, qc, pcomb_b):
                qkvt_b = qkvts[b]
                q0 = qc * QC
                for kc in range(N_TKC):
                    sp = s_ps.tile([128, HPC, QC], F32, tag="sp",
                                   name=f"sp{b}_{qc}_{kc}")
                    for h in range(HPC):
                        kt = qkvt_b[h * DK:(h + 1) * DK, 1,
                                    kc * TKC:(kc + 1) * TKC]
                        qt = qkvt_b[h * DK:(h + 1) * DK, 0, q0:q0 + QC]
                        nc.tensor.matmul(
                            sp[:, h, :], kt, qt,
                            start=True, stop=True,
                            tile_position=(h * DK, 0))
                    nc.scalar.activation(
                        pcomb_b[:, :, kc, :], sp, ActFn.Exp, scale=1.0 / 8.0)

            def emit_pv_norm(b, qc, pcomb_b):
                vp_b = vps[b]
                o65s = []
                for h in range(HPC):
                    op = o_ps.tile([128, QC], F32, tag="op",
                                   name=f"op{b}_{qc}_{h}")
                    for kc in range(N_TKC):
                        nc.tensor.matmul(
                            op[0:DK + 1, :],
                            vp_b[:, kc, h, 0:DK + 1],
                            pcomb_b[:, h, kc, :],
                            start=(kc == 0), stop=(kc == N_TKC - 1))
                    o65 = o65_pool.tile([DK + 1, QC], F32, tag="o65",
                                        name=f"o65_{b}_{qc}_{h}")
                    nc.vector.tensor_copy(o65, op[0:DK + 1, :])
                    o65s.append(o65)
                # token-major transpose, 1/rowsum, scale
                stg = stg_pool.tile([128, QC // 128, HPC, DK], BF16,
                                    tag="stg", name=f"stg{b}_{qc}")
                for h in range(HPC):
                    for r in range(QC // 128):
                        otr = tr_ps.tile([128, DK + 1], F32, tag="tr",
                                         name=f"otr{b}_{qc}_{h}_{r}")
                        nc.tensor.transpose(
                            otr, o65s[h][:, r * 128:(r + 1) * 128],
                            identity_f32[0:DK + 1, 0:DK + 1])
                        rcp = rcp_pool.tile([128, 1], F32, tag="rcp",
                                            name=f"rcp{b}_{qc}_{h}_{r}")
                        nc.vector.reciprocal(rcp, otr[:, DK:DK + 1])
                        nc.vector.tensor_scalar_mul(
                            stg[:, r, h, :], otr[:, 0:DK], rcp)
                # transpose back to feature-major and ship per-dest slabs
                sstg = sstg_pool.tile([128, QC], BF16, tag="sstg",
                                      name=f"sstg{b}_{qc}")
                for r in range(QC // 128):
                    pst = tr_ps.tile([128, 128], BF16, tag="tr",
                                     name=f"str{b}_{qc}_{r}")
                    nc.tensor.transpose(pst, stg[:, r, :, :], identity)
                    nc.vector.tensor_copy(sstg[:, r * 128:(r + 1) * 128], pst)
                for j in range(QC // TPB):
                    d = qc * (QC // TPB) + j
                    nc.sync.dma_start(
                        out=sends[b][d],
                        in_=sstg[:, j * TPB:(j + 1) * TPB])

            def emit_outproj(b):
                osb = osb_pool.tile([128, 8, TPB], BF16, tag="osb",
                                    name=f"osb{b}")
                nc.sync.dma_start(
                    out=osb, in_=recvs[b].rearrange("c p t -> p c t"))
                for tt in range(TPB // 128):
                    for ec in range(D // 512):
                        yp = y_ps.tile([128, 512], F32, tag="yp")
                        for fc in range(8):
                            nc.tensor.matmul(
                                yp,
                                osb[:, fc, tt * 128:(tt + 1) * 128],
                                wout_sb[:, fc, ec * 512:(ec + 1) * 512],
                                start=(fc == 0), stop=(fc == 7))
                        yt = yt_pool.tile([128, 512], F32, tag="yt",
                                          name=f"yt{b}_{tt}_{ec}")
                        nc.vector.tensor_add(
                            yt, yp, bout_sb[:, ec * 512:(ec + 1) * 512])
                        nc.gpsimd.dma_start(
                            out=y[b * TPB + tt * 128:b * TPB + (tt + 1) * 128,
                                  ec * 512:(ec + 1) * 512],
                            in_=yt)

            # ---------------- pipelined emission ----------------
            for st in range(4):
                emit_qkv_supertile(0, st)
            for b in range(B):
                pcombs = {}
                for qc in range(NQC):
                    pcomb_b = pcomb_pool.tile([128, HPC, N_TKC, QC], BF16,
                                              tag="pc", name=f"pc{b}_{qc}")
                    pcombs[qc] = pcomb_b
                    emit_scores_exp(b, qc, pcomb_b)
                    emit_pv_norm(b, qc, pcomb_b)
                    if b + 1 < B:
                        emit_qkv_supertile(b + 1, qc)
                nc.gpsimd.collective_compute(
                    "AllToAll",
                    AluOp.bypass,
                    replica_groups=[list(range(N_CORES))],
                    ins=[sends[b].opt()],
                    outs=[recvs[b].opt()],
                )
                emit_outproj(b)

    nc.compile()
    return nc


def _get_nc():
    if "nc" not in _CACHE:
        _CACHE["nc"] = _build()
    return _CACHE["nc"]


def kernel(x, Wqkv, bqkv, Wout, bout):
    x = np.asarray(x, dtype=np.float32)
    Wqkv = np.asarray(Wqkv, dtype=np.float32)
    bqkv = np.asarray(bqkv, dtype=np.float32)
    Wout = np.asarray(Wout, dtype=np.float32)
    bout = np.asarray(bout, dtype=np.float32)

    xtb = np.ascontiguousarray(x.reshape(T, D).T.astype(BF16NP))  # [D, T]
    woutt = np.ascontiguousarray(Wout.T.astype(BF16NP))  # [f, e]
    boutr = bout.reshape(1, D)

    in_maps = []
    for c in range(N_CORES):
        f0 = c * FPC  # first feature row of this core's heads
        rows = np.concatenate([
            Wqkv[f0:f0 + FPC],                  # q rows
            Wqkv[D + f0:D + f0 + FPC],          # k rows
            Wqkv[2 * D + f0:2 * D + f0 + FPC],  # v rows
        ])  # [384, 1024]
        wqkvt = np.ascontiguousarray(rows.T.astype(BF16NP))  # [1024, 384]
        bq = np.concatenate([
            bqkv[f0:f0 + FPC],
            bqkv[D + f0:D + f0 + FPC],
            bqkv[2 * D + f0:2 * D + f0 + FPC],
        ])  # [384]
        bqkv3 = np.ascontiguousarray(bq.reshape(3, FPC).T)  # [128, 3]
        in_maps.append({
            "xtb": xtb,
            "wqkvt": wqkvt,
            "bqkv3": bqkv3,
            "woutt": woutt,
            "boutr": boutr,
        })

    nc = _get_nc()
    trace = os.environ.get("MHA_TRACE") == "1"
    res = run_bass_kernel_spmd(
        nc, in_maps, core_ids=list(range(N_CORES)), trace=trace)
    if trace:
        _CACHE["last_result"] = res

    # y_c[b*256 + r] holds global token b*2048 + c*256 + r
    ys = np.stack([res.results[c]["y"].reshape(B, TPB, D)
                   for c in range(N_CORES)], axis=1)  # [B, core, TPB, D]
    return np.ascontiguousarray(ys.reshape(B, S, D))


# revision 7
# speedup vs baseline: 1.2846x; 1.2312x over previous
"""Multi-head attention forward, tensor-parallel over heads across 8 TRN2 cores.

Problem: B=4, S=2048, D=1024, H=16, DK=64.
  qkv = x @ Wqkv.T + bqkv ; per-head scaled-dot-product attention (no mask);
  out = attn_out @ Wout.T + bout

Sharding: 2 heads per core. Work is software-pipelined at q-chunk (512 token)
granularity: iteration i runs scores+exp(i) on PE+ACT while PV(i-1) and
normalize(i-2) interleave into the PE stream as fillers, along with the QKV
projection supertile for the NEXT batch and the output projection for tokens
received from the PREVIOUS batch's AllToAll. ACT (the exp engine) is the
critical resource and is kept saturated; everything else hides behind it.

Key structural points per core:
  - x is pre-transposed / pre-cast to bf16 on the host ([D, T]), so Q^T/K^T
    come out of the QKV matmul feature-major with no PE transposes on x.
  - V' is token-major with a fused ones-column so P@V also yields softmax
    row-sums (PSUM row 64).
  - scores: S^T[tk, tq] = K^T.T @ Q^T, two heads packed as 64-row PE tiles
    (tile_position) writing one 2-bank PSUM slab; one ACT exp op per k-chunk
    covers both heads (scale=1/sqrt(dk)); QC=512 moving dim.
  - normalize: O^T -> PE transpose -> token-major scale by 1/rowsum -> PE
    transpose back to feature-major, so AllToAll receivers can run the out
    projection directly (recv slabs are the matmul lhsT).
  - A tiny warm-up AllToAll issues at kernel start to absorb the one-time
    collective setup / cross-core start skew while early compute runs.
  - Out-proj token ownership: dest core d owns tokens [b*2048 + d*256, +256)
    for every batch b, so each batch forms a complete 8-way AllToAll that
    overlaps the next batch's attention; the host reassembles the result.
"""
import os
import sys

import numpy as np
import ml_dtypes

sys.path.insert(0, "/opt/trn_rl_repo")

import concourse.bass as bass
import concourse.mybir as mybir
import concourse.tile as tile
from concourse import bacc
from concourse.bass_utils import run_bass_kernel_spmd
from concourse.masks import make_identity

F32 = mybir.dt.float32
BF16 = mybir.dt.bfloat16
BF16NP = ml_dtypes.bfloat16

N_CORES = 8
B, S, D, H = 4, 2048, 1024, 16
DK = D // H
T = B * S  # 8192 flattened tokens
HPC = H // N_CORES  # heads per core = 2
FPC = HPC * DK  # features per core = 128
TPC = T // N_CORES  # tokens per core for out-proj = 1024
TPB = TPC // B  # out-proj tokens per core per batch = 256

QC = 512  # q-chunk (moving dim of scores / PV matmuls)
NQC = S // QC  # 4 q-chunks per batch
STT = 512  # QKV token super-tile
TKC = 128  # k-token chunk (partition dim of S^T tiles)
N_TKC = S // TKC  # 16

AluOp = mybir.AluOpType
ActFn = mybir.ActivationFunctionType

_CACHE = {}


def _build():
    nc = bacc.Bacc("TRN2", target_bir_lowering=False, debug=False,
                   num_devices=N_CORES)

    xtb = nc.dram_tensor("xtb", [D, T], BF16, kind="ExternalInput")
    wqkvt = nc.dram_tensor("wqkvt", [D, 3 * FPC], BF16, kind="ExternalInput")
    bqkv3 = nc.dram_tensor("bqkv3", [FPC, 3], F32, kind="ExternalInput")
    woutt = nc.dram_tensor("woutt", [D, D], BF16, kind="ExternalInput")
    boutr = nc.dram_tensor("boutr", [1, D], F32, kind="ExternalInput")
    y = nc.dram_tensor("y", [TPC, D], F32, kind="ExternalOutput")

    with tile.TileContext(nc) as tc:
        with (
            tc.tile_pool(name="dram", bufs=1, space="DRAM") as dram,
            tc.tile_pool(name="consts", bufs=1) as consts,
            tc.tile_pool(name="qkvt", bufs=2) as qkvt_pool,
            tc.tile_pool(name="vp", bufs=3) as vp_pool,
            tc.tile_pool(name="xt", bufs=2) as xt_pool,
            tc.tile_pool(name="pcomb", bufs=2) as pcomb_pool,
            tc.tile_pool(name="o65", bufs=4) as o65_pool,
            tc.tile_pool(name="stg", bufs=2) as stg_pool,
            tc.tile_pool(name="sstg", bufs=2) as sstg_pool,
            tc.tile_pool(name="osb", bufs=2) as osb_pool,
            tc.tile_pool(name="yt", bufs=2) as yt_pool,
            tc.tile_pool(name="rcp", bufs=4) as rcp_pool,
            tc.tile_pool(name="s_ps", bufs=2, space="PSUM") as s_ps,
            tc.tile_pool(name="o_ps", bufs=1, space="PSUM") as o_ps,
            tc.tile_pool(name="mm_ps", bufs=1, space="PSUM") as mm_ps,
            tc.tile_pool(name="tr_ps", bufs=1, space="PSUM") as tr_ps,
        ):
            sends = [dram.tile([N_CORES, FPC, TPB], BF16, name=f"send{b}")
                     for b in range(B)]
            recvs = [dram.tile([N_CORES, FPC, TPB], BF16, name=f"recv{b}")
                     for b in range(B)]
            dsend = dram.tile([N_CORES, 128], BF16, name="dsend")
            drecv = dram.tile([N_CORES, 128], BF16, name="drecv")

            identity = consts.tile([128, 128], BF16)
            make_identity(nc, identity)
            identity_f32 = consts.tile([128, 128], F32)
            make_identity(nc, identity_f32)

            w_sb = consts.tile([128, 8, 3 * FPC], BF16)  # [d_chunk, dc, f]
            nc.sync.dma_start(
                out=w_sb,
                in_=wqkvt.ap().rearrange("(dc p) f -> p dc f", p=128))
            b_sb = consts.tile([FPC, 3], F32)
            nc.sync.dma_start(out=b_sb, in_=bqkv3[:, :])
            wout_sb = consts.tile([128, 8, D], BF16)  # [f_chunk, fc, e]
            nc.sync.dma_start(
                out=wout_sb,
                in_=woutt.ap().rearrange("(fc p) e -> p fc e", p=128))
            bout_sb = consts.tile([128, D], F32)
            bout_bcast = bass.AP(
                tensor=boutr.ap().tensor,
                offset=boutr.ap().offset,
                ap=[[0, 128], boutr.ap().ap[1]])
            nc.gpsimd.dma_start(out=bout_sb, in_=bout_bcast)

            # warm-up collective: absorbs one-time CC setup + start skew
            nc.gpsimd.collective_compute(
                "AllToAll", AluOp.bypass,
                replica_groups=[list(range(N_CORES))],
                ins=[dsend.opt()], outs=[drecv.opt()])

            xtb_r = xtb.ap().rearrange("(dc p) t -> p dc t", p=128)

            qkvts = {}
            vps = {}

            # ---------- filler thunks (PE work interleaved into kc slots) ----
            def qkv_supertile_thunks(b, st):
                # QKV projection for batch b, tokens [st*512, (st+1)*512)
                if b not in qkvts:
                    qkvts[b] = qkvt_pool.tile([128, 3, S], BF16, tag="qkvt",
                                              name=f"qkvt{b}")
                    vps[b] = vp_pool.tile([128, N_TKC, HPC, 66], BF16,
                                          tag="vp", name=f"vp{b}")
                    nc.vector.memset(vps[b][:, :, :, 64:65], 1.0)
                qkvt_b, vp_b = qkvts[b], vps[b]
                t0 = b * S + st * STT
                xt = xt_pool.tile([128, 8, STT], BF16, tag="xt",
                                  name=f"xt{b}_{st}")

                def load():
                    nc.sync.dma_start(out=xt, in_=xtb_r[:, :, t0:t0 + STT])

                def proj(fc):
                    ps = mm_ps.tile([128, STT], F32, tag="mm",
                                    name=f"qps{b}_{st}_{fc}")
                    for dc in range(8):
                        nc.tensor.matmul(
                            ps,
                            w_sb[:, dc, fc * FPC:(fc + 1) * FPC],
                            xt[:, dc, :],
                            start=(dc == 0), stop=(dc == 7))
                    nc.vector.tensor_scalar_add(
                        qkvt_b[:, fc, st * STT:(st + 1) * STT], ps,
                        b_sb[:, fc:fc + 1])

                def vprep(kc):
                    pst = tr_ps.tile([128, 128], BF16, tag="tr",
                                     name=f"vtr{b}_{kc}")
                    nc.tensor.transpose(
                        pst, qkvt_b[:, 2, kc * TKC:(kc + 1) * TKC], identity)
                    nc.vector.tensor_copy(
                        vp_b[:, kc, :, 0:DK],
                        pst.rearrange("p (h k) -> p h k", h=HPC))

                thunks = [load]
                thunks += [lambda fc=fc: proj(fc) for fc in range(3)]
                thunks += [lambda kc=kc: vprep(kc)
                           for kc in range(st * (STT // TKC),
                                           (st + 1) * (STT // TKC))]
                return thunks

            def norm_thunks(b, qc, o65s):
                # normalize + feature-major send for q-chunk (b, qc)
                stg = stg_pool.tile([128, QC // 128, HPC, DK], BF16,
                                    tag="stg", name=f"stg{b}_{qc}")
                sstg = sstg_pool.tile([128, QC], BF16, tag="sstg",
                                      name=f"sstg{b}_{qc}")

                def onorm(h, r):
                    otr = tr_ps.tile([128, DK + 1], F32, tag="tr",
                                     name=f"otr{b}_{qc}_{h}_{r}")
                    nc.tensor.transpose(
                        otr, o65s[h][:, r * 128:(r + 1) * 128],
                        identity_f32[0:DK + 1, 0:DK + 1])
                    rcp = rcp_pool.tile([128, 1], F32, tag="rcp",
                                        name=f"rcp{b}_{qc}_{h}_{r}")
                    nc.vector.reciprocal(rcp, otr[:, DK:DK + 1])
                    nc.vector.tensor_scalar_mul(
                        stg[:, r, h, :], otr[:, 0:DK], rcp)

                def sendtr(r):
                    pst = tr_ps.tile([128, 128], BF16, tag="tr",
                                     name=f"str{b}_{qc}_{r}")
                    nc.tensor.transpose(pst, stg[:, r, :, :], identity)
                    nc.vector.tensor_copy(sstg[:, r * 128:(r + 1) * 128], pst)

                def ship(j):
                    d = qc * (QC // TPB) + j
                    nc.sync.dma_start(
                        out=sends[b][d],
                        in_=sstg[:, j * TPB:(j + 1) * TPB])

                thunks = [lambda h=h, r=r: onorm(h, r)
                          for h in range(HPC) for r in range(QC // 128)]
                thunks += [lambda r=r: sendtr(r) for r in range(QC // 128)]
                thunks += [lambda j=j: ship(j) for j in range(QC // TPB)]
                return thunks

            def outproj_thunks(b):
                osb = osb_pool.tile([128, 8, TPB], BF16, tag="osb",
                                    name=f"osb{b}")

                def load():
                    nc.sync.dma_start(
                        out=osb, in_=recvs[b].rearrange("c p t -> p c t"))

                def chunk(tt, ec):
                    yp = mm_ps.tile([128, 512], F32, tag="mm",
                                    name=f"yp{b}_{tt}_{ec}")
                    for fc in range(8):
                        nc.tensor.matmul(
                            yp,
                            osb[:, fc, tt * 128:(tt + 1) * 128],
                            wout_sb[:, fc, ec * 512:(ec + 1) * 512],
                            start=(fc == 0), stop=(fc == 7))
                    yt = yt_pool.tile([128, 512], F32, tag="yt",
                                      name=f"yt{b}_{tt}_{ec}")
                    nc.vector.tensor_add(
                        yt, yp, bout_sb[:, ec * 512:(ec + 1) * 512])
                    nc.sync.dma_start(
                        out=y[b * TPB + tt * 128:b * TPB + (tt + 1) * 128,
                              ec * 512:(ec + 1) * 512],
                        in_=yt)

                return [load] + [lambda tt=tt, ec=ec: chunk(tt, ec)
                                 for tt in range(TPB // 128)
                                 for ec in range(D // 512)]

            # ---------- main per-iteration emission ----------
            def emit_iter(cur, pv_st, fillers):
                """cur=(b,qc) scores+exp; pv_st=(b,qc,pcomb,o65s) PV chains
                interleaved per kc; fillers: list of thunks to spread."""
                b, qc = cur
                qkvt_b = qkvts[b]
                q0 = qc * QC
                pcomb = pcomb_pool.tile([128, HPC, N_TKC, QC], BF16,
                                        tag="pc", name=f"pc{b}_{qc}")
                ops = None
                if pv_st is not None:
                    pb, pqc, ppcomb, po65s = pv_st
                    ops = o_ps.tile([128, HPC, QC], F32, tag="op",
                                    name=f"op{pb}_{pqc}")
                fq = list(fillers)
                fi = 0
                for kc in range(N_TKC):
                    sp = s_ps.tile([128, HPC, QC], F32, tag="sp",
                                   name=f"sp{b}_{qc}_{kc}")
                    for h in range(HPC):
                        kt = qkvt_b[h * DK:(h + 1) * DK, 1,
                                    kc * TKC:(kc + 1) * TKC]
                        qt = qkvt_b[h * DK:(h + 1) * DK, 0, q0:q0 + QC]
                        nc.tensor.matmul(
                            sp[:, h, :], kt, qt,
                            start=True, stop=True,
                            tile_position=(h * DK, 0))
                    nc.scalar.activation(
                        pcomb[:, :, kc, :], sp, ActFn.Exp, scale=1.0 / 8.0)
                    if pv_st is not None:
                        pb, pqc, ppcomb, po65s = pv_st
                        vp_p = vps[pb]
                        for h in range(HPC):
                            nc.tensor.matmul(
                                ops[0:DK + 1, h, :],
                                vp_p[:, kc, h, 0:DK + 1],
                                ppcomb[:, h, kc, :],
                                start=(kc == 0), stop=(kc == N_TKC - 1))
                    # spread filler thunks proportionally across kc slots
                    while fi < len(fq) and fi * N_TKC <= (kc + 1) * len(fq) - N_TKC:
                        fq[fi]()
                        fi += 1
                # evacuate PV results (frees o_ps for the next iteration)
                o65s = None
                if pv_st is not None:
                    pb, pqc, _, _ = pv_st
                    o65s = []
                    for h in range(HPC):
                        o65 = o65_pool.tile([DK + 1, QC], F32, tag="o65",
                                            name=f"o65_{pb}_{pqc}_{h}")
                        nc.vector.tensor_copy(o65, ops[0:DK + 1, h, :])
                        o65s.append(o65)
                # leftover fillers
                while fi < len(fq):
                    fq[fi]()
                    fi += 1
                return pcomb, o65s

            # ---------- pipeline ----------
            for st in range(4):
                for t in qkv_supertile_thunks(0, st):
                    t()

            # iteration stream: (b, qc) for all batches
            iters = [(b, qc) for b in range(B) for qc in range(NQC)]
            pv_st = None      # (b, qc, pcomb, o65s) awaiting PV
            nm_st = None      # (b, qc, o65s) awaiting normalize
            for idx, (b, qc) in enumerate(iters):
                fillers = []
                if nm_st is not None:
                    nb, nqc, no65s = nm_st
                    fillers += norm_thunks(nb, nqc, no65s)
                    if nqc == NQC - 1:
                        # last q-chunk of batch nb normalized -> collective
                        fillers += [lambda nb=nb: nc.gpsimd.collective_compute(
                            "AllToAll", AluOp.bypass,
                            replica_groups=[list(range(N_CORES))],
                            ins=[sends[nb].opt()], outs=[recvs[nb].opt()])]
                    if nqc == 0 and nb >= 1:
                        fillers += outproj_thunks(nb - 1)
                if b + 1 < B:
                    fillers += qkv_supertile_thunks(b + 1, qc)
                pcomb, o65s = emit_iter((b, qc), pv_st, fillers)
                if pv_st is not None:
                    pb, pqc, ppcomb, _ = pv_st
                    nm_st = (pb, pqc, o65s)
                pv_st = (b, qc, pcomb, None)

            # ---------- epilogue ----------
            # PV for the last q-chunk (dense; all exps done)
            b, qc = iters[-1]
            ops = o_ps.tile([128, HPC, QC], F32, tag="op", name="op_last")
            vp_b = vps[b]
            for h in range(HPC):
                for kc in range(N_TKC):
                    nc.tensor.matmul(
                        ops[0:DK + 1, h, :],
                        vp_b[:, kc, h, 0:DK + 1],
                        pv_st[2][:, h, kc, :],
                        start=(kc == 0), stop=(kc == N_TKC - 1))
            o65s = []
            for h in range(HPC):
                o65 = o65_pool.tile([DK + 1, QC], F32, tag="o65",
                                    name=f"o65_last_{h}")
                nc.vector.tensor_copy(o65, ops[0:DK + 1, h, :])
                o65s.append(o65)
            # normalize (b,2) then (b,3), ship, final collective + outproj
            nb, nqc, no65s = nm_st
            for t in norm_thunks(nb, nqc, no65s):
                t()
            for t in norm_thunks(b, qc, o65s):
                t()
            nc.gpsimd.collective_compute(
                "AllToAll", AluOp.bypass,
                replica_groups=[list(range(N_CORES))],
                ins=[sends[b].opt()], outs=[recvs[b].opt()])
            for t in outproj_thunks(B - 1):
                t()

    nc.compile()
    return nc


def _get_nc():
    if "nc" not in _CACHE:
        _CACHE["nc"] = _build()
    return _CACHE["nc"]


def kernel(x, Wqkv, bqkv, Wout, bout):
    x = np.asarray(x, dtype=np.float32)
    Wqkv = np.asarray(Wqkv, dtype=np.float32)
    bqkv = np.asarray(bqkv, dtype=np.float32)
    Wout = np.asarray(Wout, dtype=np.float32)
    bout = np.asarray(bout, dtype=np.float32)

    xtb = np.ascontiguousarray(x.reshape(T, D).T.astype(BF16NP))  # [D, T]
    woutt = np.ascontiguousarray(Wout.T.astype(BF16NP))  # [f, e]
    boutr = bout.reshape(1, D)

    in_maps = []
    for c in range(N_CORES):
        f0 = c * FPC  # first feature row of this core's heads
        rows = np.concatenate([
            Wqkv[f0:f0 + FPC],                  # q rows
            Wqkv[D + f0:D + f0 + FPC],          # k rows
            Wqkv[2 * D + f0:2 * D + f0 + FPC],  # v rows
        ])  # [384, 1024]
        wqkvt = np.ascontiguousarray(rows.T.astype(BF16NP))  # [1024, 384]
        bq = np.concatenate([
            bqkv[f0:f0 + FPC],
            bqkv[D + f0:D + f0 + FPC],
            bqkv[2 * D + f0:2 * D + f0 + FPC],
        ])  # [384]
        bqkv3 = np.ascontiguousarray(bq.reshape(3, FPC).T)  # [128, 3]
        in_maps.append({
            "xtb": xtb,
            "wqkvt": wqkvt,
            "bqkv3": bqkv3,
            "woutt": woutt,
            "boutr": boutr,
        })

    nc = _get_nc()
    trace = os.environ.get("MHA_TRACE") == "1"
    res = run_bass_kernel_spmd(
        nc, in_maps, core_ids=list(range(N_CORES)), trace=trace)
    if trace:
        _CACHE["last_result"] = res

    # y_c[b*256 + r] holds global token b*2048 + c*256 + r
    ys = np.stack([res.results[c]["y"].reshape(B, TPB, D)
                   for c in range(N_CORES)], axis=1)  # [B, core, TPB, D]
    return np.ascontiguousarray(ys.reshape(B, S, D))


# revision 13
# speedup vs baseline: 1.4014x; 1.0909x over previous
"""Multi-head attention forward, tensor-parallel over heads across 8 TRN2 cores.

Problem: B=4, S=2048, D=1024, H=16, DK=64.
  qkv = x @ Wqkv.T + bqkv ; per-head scaled-dot-product attention (no mask);
  out = attn_out @ Wout.T + bout

Sharding: 2 heads per core. Work is software-pipelined at q-chunk (512 token)
granularity: iteration i runs scores+exp(i) on PE+ACT while PV(i-1) and
normalize(i-2) interleave into the PE stream as fillers, along with the QKV
projection supertile for the NEXT batch and the output projection for tokens
received from the PREVIOUS batch's AllToAll. ACT (the exp engine) is the
critical resource and is kept saturated; everything else hides behind it.

Key structural points per core:
  - x is pre-transposed / pre-cast to bf16 on the host ([D, T]), so Q^T/K^T
    come out of the QKV matmul feature-major with no PE transposes on x.
  - V' is token-major with a fused ones-column so P@V also yields softmax
    row-sums (PSUM row 64).
  - scores: S^T[tk, tq] = K^T.T @ Q^T, two heads packed as 64-row PE tiles
    (tile_position) writing one 2-bank PSUM slab; one ACT exp op per k-chunk
    covers both heads (scale=1/sqrt(dk)); QC=512 moving dim.
  - normalize: O^T -> PE transpose -> token-major scale by 1/rowsum -> PE
    transpose back to feature-major, so AllToAll receivers can run the out
    projection directly (recv slabs are the matmul lhsT).
  - A tiny warm-up AllToAll issues at kernel start to absorb the one-time
    collective setup / cross-core start skew while early compute runs.
  - Out-proj token ownership: dest core d owns tokens [b*2048 + d*256, +256)
    for every batch b, so each batch forms a complete 8-way AllToAll that
    overlaps the next batch's attention; the host reassembles the result.
"""
import os
import sys

import numpy as np
import ml_dtypes

sys.path.insert(0, "/opt/trn_rl_repo")

import concourse.bass as bass
import concourse.mybir as mybir
import concourse.tile as tile
from concourse import bacc
from concourse.bass_utils import run_bass_kernel_spmd
from concourse.masks import make_identity

F32 = mybir.dt.float32
BF16 = mybir.dt.bfloat16
BF16NP = ml_dtypes.bfloat16

N_CORES = 8
B, S, D, H = 4, 2048, 1024, 16
DK = D // H
T = B * S  # 8192 flattened tokens
HPC = H // N_CORES  # heads per core = 2
FPC = HPC * DK  # features per core = 128
TPC = T // N_CORES  # tokens per core for out-proj = 1024
TPB = TPC // B  # out-proj tokens per core per batch = 256

QC = 512  # q-chunk (moving dim of scores / PV matmuls)
NQC = S // QC  # 4 q-chunks per batch
STT = 512  # QKV token super-tile
TKC = 128  # k-token chunk (partition dim of S^T tiles)
N_TKC = S // TKC  # 16

AluOp = mybir.AluOpType
ActFn = mybir.ActivationFunctionType

_CACHE = {}


def _build():
    nc = bacc.Bacc("TRN2", target_bir_lowering=False, debug=False,
                   num_devices=N_CORES)

    xtb = nc.dram_tensor("xtb", [D, T], BF16, kind="ExternalInput")
    wqkvt = nc.dram_tensor("wqkvt", [D, 3 * FPC], BF16, kind="ExternalInput")
    bqkv3 = nc.dram_tensor("bqkv3", [FPC, 3], F32, kind="ExternalInput")
    woutt = nc.dram_tensor("woutt", [D, D], BF16, kind="ExternalInput")
    boutr = nc.dram_tensor("boutr", [1, D], F32, kind="ExternalInput")
    y = nc.dram_tensor("y", [TPC, D], F32, kind="ExternalOutput")

    with tile.TileContext(nc) as tc:
        with (
            tc.tile_pool(name="dram", bufs=1, space="DRAM") as dram,
            tc.tile_pool(name="consts", bufs=1) as consts,
            tc.tile_pool(name="qkvt", bufs=2) as qkvt_pool,
            tc.tile_pool(name="vp", bufs=3) as vp_pool,
            tc.tile_pool(name="xt", bufs=2) as xt_pool,
            tc.tile_pool(name="pcomb", bufs=2) as pcomb_pool,
            tc.tile_pool(name="stg", bufs=2) as stg_pool,
            tc.tile_pool(name="sstg", bufs=2) as sstg_pool,
            tc.tile_pool(name="osb", bufs=2) as osb_pool,
            tc.tile_pool(name="yt", bufs=2) as yt_pool,
            tc.tile_pool(name="rcp", bufs=4) as rcp_pool,
            tc.tile_pool(name="s_ps", bufs=2, space="PSUM") as s_ps,
            tc.tile_pool(name="o_ps", bufs=1, space="PSUM") as o_ps,
            tc.tile_pool(name="mm_ps", bufs=1, space="PSUM") as mm_ps,
            tc.tile_pool(name="tr_ps", bufs=1, space="PSUM") as tr_ps,
        ):
            sends = [dram.tile([N_CORES, FPC, TPB], BF16, name=f"send{b}")
                     for b in range(B)]
            recvs = [dram.tile([N_CORES, FPC, TPB], BF16, name=f"recv{b}")
                     for b in range(B)]
            dsend = dram.tile([N_CORES, 128], BF16, name="dsend")
            drecv = dram.tile([N_CORES, 128], BF16, name="drecv")

            identity = consts.tile([128, 128], BF16)
            make_identity(nc, identity)

            w_sb = consts.tile([128, 8, 3 * FPC], BF16)  # [d_chunk, dc, f]
            nc.sync.dma_start(
                out=w_sb,
                in_=wqkvt.ap().rearrange("(dc p) f -> p dc f", p=128))
            b_sb = consts.tile([FPC, 3], F32)
            nc.sync.dma_start(out=b_sb, in_=bqkv3[:, :])
            wout_sb = consts.tile([128, 8, D], BF16)  # [f_chunk, fc, e]
            nc.scalar.dma_start(
                out=wout_sb,
                in_=woutt.ap().rearrange("(fc p) e -> p fc e", p=128))
            bout_sb = consts.tile([128, D], F32)
            bout_bcast = bass.AP(
                tensor=boutr.ap().tensor,
                offset=boutr.ap().offset,
                ap=[[0, 128], boutr.ap().ap[1]])
            nc.gpsimd.dma_start(out=bout_sb, in_=bout_bcast)

            # warm-up collective: absorbs one-time CC setup + start skew
            nc.gpsimd.collective_compute(
                "AllToAll", AluOp.bypass,
                replica_groups=[list(range(N_CORES))],
                ins=[dsend.opt()], outs=[drecv.opt()])

            xtb_r = xtb.ap().rearrange("(dc p) t -> p dc t", p=128)

            qkvts = {}
            vps = {}

            # ---------- filler thunks (PE work interleaved into kc slots) ----
            def qkv_supertile_thunks(b, st):
                # QKV projection for batch b, tokens [st*512, (st+1)*512)
                if b not in qkvts:
                    qkvts[b] = qkvt_pool.tile([128, 3, S], BF16, tag="qkvt",
                                              name=f"qkvt{b}")
                    vps[b] = vp_pool.tile([128, N_TKC, HPC, 66], BF16,
                                          tag="vp", name=f"vp{b}")
                    nc.vector.memset(vps[b][:, :, :, 64:65], 1.0)
                qkvt_b, vp_b = qkvts[b], vps[b]
                t0 = b * S + st * STT
                xt = xt_pool.tile([128, 8, STT], BF16, tag="xt",
                                  name=f"xt{b}_{st}")

                def load():
                    nc.sync.dma_start(out=xt, in_=xtb_r[:, :, t0:t0 + STT])

                def proj(fc):
                    ps = mm_ps.tile([128, STT], F32, tag="mm",
                                    name=f"qps{b}_{st}_{fc}")
                    for dc in range(8):
                        nc.tensor.matmul(
                            ps,
                            w_sb[:, dc, fc * FPC:(fc + 1) * FPC],
                            xt[:, dc, :],
                            start=(dc == 0), stop=(dc == 7))
                    nc.vector.tensor_scalar_add(
                        qkvt_b[:, fc, st * STT:(st + 1) * STT], ps,
                        b_sb[:, fc:fc + 1])

                def vprep(kc):
                    pst = tr_ps.tile([128, 128], BF16, tag="tr",
                                     name=f"vtr{b}_{kc}")
                    nc.tensor.transpose(
                        pst, qkvt_b[:, 2, kc * TKC:(kc + 1) * TKC], identity)
                    nc.vector.tensor_copy(
                        vp_b[:, kc, :, 0:DK],
                        pst.rearrange("p (h k) -> p h k", h=HPC))

                thunks = [load]
                thunks += [lambda fc=fc: proj(fc) for fc in range(3)]
                thunks += [lambda kc=kc: vprep(kc)
                           for kc in range(st * (STT // TKC),
                                           (st + 1) * (STT // TKC))]
                return thunks

            def pv_tail(b, qc, ops):
                # normalize token-major PV output straight out of PSUM:
                # ops[:, h, tc*65+64] holds the softmax row-sum
                stg = stg_pool.tile([128, QC // 128, HPC, DK], BF16,
                                    tag="stg", name=f"stg{b}_{qc}")
                for h in range(HPC):
                    for tc in range(QC // 128):
                        rcp = rcp_pool.tile([128, 1], F32, tag="rcp",
                                            name=f"rcp{b}_{qc}_{h}_{tc}")
                        nc.vector.reciprocal(
                            rcp, ops[:, h, tc * 65 + DK:tc * 65 + DK + 1])
                        nc.vector.tensor_scalar_mul(
                            stg[:, tc, h, :], ops[:, h, tc * 65:tc * 65 + DK],
                            rcp)
                return stg

            def norm_thunks(b, qc, stg):
                # feature-major transpose-back + send for q-chunk (b, qc)
                sstg = sstg_pool.tile([128, QC], BF16, tag="sstg",
                                      name=f"sstg{b}_{qc}")

                def sendtr(r):
                    pst = tr_ps.tile([128, 128], BF16, tag="tr",
                                     name=f"str{b}_{qc}_{r}")
                    nc.tensor.transpose(pst, stg[:, r, :, :], identity)
                    nc.vector.tensor_copy(sstg[:, r * 128:(r + 1) * 128], pst)

                def ship(j):
                    d = qc * (QC // TPB) + j
                    nc.sync.dma_start(
                        out=sends[b][d],
                        in_=sstg[:, j * TPB:(j + 1) * TPB])

                thunks = [lambda r=r: sendtr(r) for r in range(QC // 128)]
                thunks += [lambda j=j: ship(j) for j in range(QC // TPB)]
                return thunks

            def outproj_thunks(b):
                osb = osb_pool.tile([128, 8, TPB], BF16, tag="osb",
                                    name=f"osb{b}")

                def load():
                    nc.sync.dma_start(
                        out=osb, in_=recvs[b].rearrange("c p t -> p c t"))

                def chunk(tt, ec):
                    yp = mm_ps.tile([128, 512], F32, tag="mm",
                                    name=f"yp{b}_{tt}_{ec}")
                    for fc in range(8):
                        nc.tensor.matmul(
                            yp,
                            osb[:, fc, tt * 128:(tt + 1) * 128],
                            wout_sb[:, fc, ec * 512:(ec + 1) * 512],
                            start=(fc == 0), stop=(fc == 7))
                    yt = yt_pool.tile([128, 512], F32, tag="yt",
                                      name=f"yt{b}_{tt}_{ec}")
                    nc.vector.tensor_add(
                        yt, yp, bout_sb[:, ec * 512:(ec + 1) * 512])
                    nc.sync.dma_start(
                        out=y[b * TPB + tt * 128:b * TPB + (tt + 1) * 128,
                              ec * 512:(ec + 1) * 512],
                        in_=yt)

                return [load] + [lambda tt=tt, ec=ec: chunk(tt, ec)
                                 for tt in range(TPB // 128)
                                 for ec in range(D // 512)]

            # ---------- main per-iteration emission ----------
            def emit_pv_mms(ops, ppcomb, vp_p, kc):
                # PV with pcomb stationary: O[tq, dk+1] token-major, N=65.
                # 8 chains (h x tc) share 2 banks; exactly one start=True per
                # bank marks the whole zero-region pending (HW has_written
                # semantics), every other first-touch overwrites, later MMs
                # accumulate.
                for h in range(HPC):
                    for tc in range(QC // 128):
                        nc.tensor.matmul(
                            ops[:, h, tc * 65:tc * 65 + DK + 1],
                            ppcomb[:, h, kc, tc * 128:(tc + 1) * 128],
                            vp_p[:, kc, h, 0:DK + 1],
                            start=(kc == 0 and tc == 0),
                            stop=(kc == N_TKC - 1),
                            skip_group_check=True)

            def emit_iter(cur, pv_st, fillers):
                """cur=(b,qc) scores+exp; pv_st=(b,qc,pcomb) PV chains
                interleaved per kc; fillers: list of thunks to spread."""
                b, qc = cur
                qkvt_b = qkvts[b]
                q0 = qc * QC
                pcomb = pcomb_pool.tile([128, HPC, N_TKC, QC], BF16,
                                        tag="pc", name=f"pc{b}_{qc}")
                ops = None
                if pv_st is not None:
                    pb, pqc, ppcomb = pv_st
                    vp_p = vps[pb]
                    ops = o_ps.tile([128, HPC, QC], F32, tag="op",
                                    name=f"op{pb}_{pqc}")
                fq = list(fillers)
                fi = 0
                for kc in range(N_TKC):
                    sp = s_ps.tile([128, HPC, QC], F32, tag="sp",
                                   name=f"sp{b}_{qc}_{kc}")
                    for h in range(HPC):
                        kt = qkvt_b[h * DK:(h + 1) * DK, 1,
                                    kc * TKC:(kc + 1) * TKC]
                        qt = qkvt_b[h * DK:(h + 1) * DK, 0, q0:q0 + QC]
                        nc.tensor.matmul(
                            sp[:, h, :], kt, qt,
                            start=True, stop=True,
                            tile_position=(h * DK, 0))
                    nc.scalar.activation(
                        pcomb[:, :, kc, :], sp, ActFn.Exp, scale=1.0 / 8.0)
                    if pv_st is not None:
                        emit_pv_mms(ops, ppcomb, vp_p, kc)
                    # spread filler thunks proportionally across kc slots
                    while fi < len(fq) and fi * N_TKC <= (kc + 1) * len(fq) - N_TKC:
                        fq[fi]()
                        fi += 1
                # normalize PV output, freeing o_ps for the next iteration
                stg = None
                if pv_st is not None:
                    pb, pqc, _ = pv_st
                    stg = pv_tail(pb, pqc, ops)
                # leftover fillers
                while fi < len(fq):
                    fq[fi]()
                    fi += 1
                return pcomb, stg

            # ---------- pipeline ----------
            for st in range(4):
                for t in qkv_supertile_thunks(0, st):
                    t()

            # iteration stream: (b, qc) for all batches
            iters = [(b, qc) for b in range(B) for qc in range(NQC)]
            pv_st = None      # (b, qc, pcomb) awaiting PV
            nm_st = None      # (b, qc, stg) awaiting transpose-back + send
            for idx, (b, qc) in enumerate(iters):
                fillers = []
                if nm_st is not None:
                    nb, nqc, nstg = nm_st
                    fillers += norm_thunks(nb, nqc, nstg)
                    if nqc == NQC - 1:
                        # last q-chunk of batch nb shipped -> collective
                        fillers += [lambda nb=nb: nc.gpsimd.collective_compute(
                            "AllToAll", AluOp.bypass,
                            replica_groups=[list(range(N_CORES))],
                            ins=[sends[nb].opt()], outs=[recvs[nb].opt()])]
                    if nqc == 0 and nb >= 1:
                        fillers += outproj_thunks(nb - 1)
                if b + 1 < B:
                    fillers += qkv_supertile_thunks(b + 1, qc)
                pcomb, stg = emit_iter((b, qc), pv_st, fillers)
                if pv_st is not None:
                    pb, pqc, _ = pv_st
                    nm_st = (pb, pqc, stg)
                pv_st = (b, qc, pcomb)

            # ---------- epilogue ----------
            # PV for the last q-chunk (dense; all exps done)
            b, qc = iters[-1]
            ops = o_ps.tile([128, HPC, QC], F32, tag="op", name="op_last")
            for kc in range(N_TKC):
                emit_pv_mms(ops, pv_st[2], vps[b], kc)
            stg_last = pv_tail(b, qc, ops)
            # ship (b,2) then (b,3), final collective + outproj
            nb, nqc, nstg = nm_st
            for t in norm_thunks(nb, nqc, nstg):
                t()
            for t in norm_thunks(b, qc, stg_last):
                t()
            nc.gpsimd.collective_compute(
                "AllToAll", AluOp.bypass,
                replica_groups=[list(range(N_CORES))],
                ins=[sends[b].opt()], outs=[recvs[b].opt()])
            for t in outproj_thunks(B - 1):
                t()

    nc.compile()
    return nc


def _get_nc():
    if "nc" not in _CACHE:
        _CACHE["nc"] = _build()
    return _CACHE["nc"]


def kernel(x, Wqkv, bqkv, Wout, bout):
    x = np.asarray(x, dtype=np.float32)
    Wqkv = np.asarray(Wqkv, dtype=np.float32)
    bqkv = np.asarray(bqkv, dtype=np.float32)
    Wout = np.asarray(Wout, dtype=np.float32)
    bout = np.asarray(bout, dtype=np.float32)

    xtb = np.ascontiguousarray(x.reshape(T, D).T.astype(BF16NP))  # [D, T]
    woutt = np.ascontiguousarray(Wout.T.astype(BF16NP))  # [f, e]
    boutr = bout.reshape(1, D)

    in_maps = []
    for c in range(N_CORES):
        f0 = c * FPC  # first feature row of this core's heads
        rows = np.concatenate([
            Wqkv[f0:f0 + FPC],                  # q rows
            Wqkv[D + f0:D + f0 + FPC],          # k rows
            Wqkv[2 * D + f0:2 * D + f0 + FPC],  # v rows
        ])  # [384, 1024]
        wqkvt = np.ascontiguousarray(rows.T.astype(BF16NP))  # [1024, 384]
        bq = np.concatenate([
            bqkv[f0:f0 + FPC],
            bqkv[D + f0:D + f0 + FPC],
            bqkv[2 * D + f0:2 * D + f0 + FPC],
        ])  # [384]
        bqkv3 = np.ascontiguousarray(bq.reshape(3, FPC).T)  # [128, 3]
        in_maps.append({
            "xtb": xtb,
            "wqkvt": wqkvt,
            "bqkv3": bqkv3,
            "woutt": woutt,
            "boutr": boutr,
        })

    nc = _get_nc()
    trace = os.environ.get("MHA_TRACE") == "1"
    res = run_bass_kernel_spmd(
        nc, in_maps, core_ids=list(range(N_CORES)), trace=trace)
    if trace:
        _CACHE["last_result"] = res

    # y_c[b*256 + r] holds global token b*2048 + c*256 + r
    ys = np.stack([res.results[c]["y"].reshape(B, TPB, D)
                   for c in range(N_CORES)], axis=1)  # [B, core, TPB, D]
    return np.ascontiguousarray(ys.reshape(B, S, D))


# revision 18
# speedup vs baseline: 1.5107x; 1.0780x over previous
"""Multi-head attention forward, tensor-parallel over heads across 8 TRN2 cores.

Problem: B=4, S=2048, D=1024, H=16, DK=64.
  qkv = x @ Wqkv.T + bqkv ; per-head scaled-dot-product attention (no mask);
  out = attn_out @ Wout.T + bout

Sharding: 2 heads per core. Work is software-pipelined at q-chunk (512 token)
granularity: iteration i runs scores+exp(i) on PE+ACT while PV(i-1) and
normalize(i-2) interleave into the PE stream as fillers, along with the QKV
projection supertile for the NEXT batch and the output projection for tokens
received from the PREVIOUS batch's AllToAll. ACT (the exp engine) is the
critical resource and is kept saturated; everything else hides behind it.

Key structural points per core:
  - x is pre-transposed / pre-cast to bf16 on the host ([D, T]), so Q^T/K^T
    come out of the QKV matmul feature-major with no PE transposes on x.
  - V' is token-major with a fused ones-column so P@V also yields softmax
    row-sums (PSUM row 64).
  - scores: S^T[tk, tq] = K^T.T @ Q^T, two heads packed as 64-row PE tiles
    (tile_position) writing one 2-bank PSUM slab; one ACT exp op per k-chunk
    covers both heads (scale=1/sqrt(dk)); QC=512 moving dim.
  - normalize: O^T -> PE transpose -> token-major scale by 1/rowsum -> PE
    transpose back to feature-major, so AllToAll receivers can run the out
    projection directly (recv slabs are the matmul lhsT).
  - A tiny warm-up AllToAll issues at kernel start to absorb the one-time
    collective setup / cross-core start skew while early compute runs.
  - Out-proj token ownership: dest core d owns tokens [b*2048 + d*256, +256)
    for every batch b, so each batch forms a complete 8-way AllToAll that
    overlaps the next batch's attention; the host reassembles the result.
"""
import os
import sys

import numpy as np
import ml_dtypes

sys.path.insert(0, "/opt/trn_rl_repo")

import concourse.bass as bass
import concourse.mybir as mybir
import concourse.tile as tile
from concourse import bacc
from concourse.bass_utils import run_bass_kernel_spmd
from concourse.masks import make_identity

F32 = mybir.dt.float32
BF16 = mybir.dt.bfloat16
BF16NP = ml_dtypes.bfloat16

N_CORES = 8
B, S, D, H = 4, 2048, 1024, 16
DK = D // H
T = B * S  # 8192 flattened tokens
HPC = H // N_CORES  # heads per core = 2
FPC = HPC * DK  # features per core = 128
TPC = T // N_CORES  # tokens per core for out-proj = 1024
TPB = TPC // B  # out-proj tokens per core per batch = 256

QC = 512  # q-chunk (moving dim of scores / PV matmuls)
NQC = S // QC  # 4 q-chunks per batch
STT = 512  # QKV token super-tile
TKC = 128  # k-token chunk (partition dim of S^T tiles)
N_TKC = S // TKC  # 16

AluOp = mybir.AluOpType
ActFn = mybir.ActivationFunctionType

_CACHE = {}


def _build():
    nc = bacc.Bacc("TRN2", target_bir_lowering=False, debug=False,
                   num_devices=N_CORES)

    xtb = nc.dram_tensor("xtb", [D, T], BF16, kind="ExternalInput")
    wqkvt = nc.dram_tensor("wqkvt", [D, 3 * FPC], BF16, kind="ExternalInput")
    bqkv3 = nc.dram_tensor("bqkv3", [FPC, 3], F32, kind="ExternalInput")
    woutt = nc.dram_tensor("woutt", [D, D], BF16, kind="ExternalInput")
    boutr = nc.dram_tensor("boutr", [1, D], F32, kind="ExternalInput")
    y = nc.dram_tensor("y", [TPC, D], F32, kind="ExternalOutput")

    with tile.TileContext(nc) as tc:
        with (
            tc.tile_pool(name="dram", bufs=1, space="DRAM") as dram,
            tc.tile_pool(name="consts", bufs=1) as consts,
            tc.tile_pool(name="qkvt", bufs=2) as qkvt_pool,
            tc.tile_pool(name="vp", bufs=3) as vp_pool,
            tc.tile_pool(name="xt", bufs=2) as xt_pool,
            tc.tile_pool(name="pcomb", bufs=2) as pcomb_pool,
            tc.tile_pool(name="stg", bufs=2) as stg_pool,
            tc.tile_pool(name="sstg", bufs=2) as sstg_pool,
            tc.tile_pool(name="osb", bufs=2) as osb_pool,
            tc.tile_pool(name="yt", bufs=2) as yt_pool,
            tc.tile_pool(name="rcp", bufs=4) as rcp_pool,
            tc.tile_pool(name="s_ps", bufs=2, space="PSUM") as s_ps,
            tc.tile_pool(name="o_ps", bufs=1, space="PSUM") as o_ps,
            tc.tile_pool(name="mm_ps", bufs=1, space="PSUM") as mm_ps,
            tc.tile_pool(name="tr_ps", bufs=1, space="PSUM") as tr_ps,
        ):
            sends = [dram.tile([N_CORES, FPC, TPB], BF16, name=f"send{b}")
                     for b in range(B)]
            recvs = [dram.tile([N_CORES, FPC, TPB], BF16, name=f"recv{b}")
                     for b in range(B)]
            dsend = dram.tile([N_CORES, FPC, TPB], BF16, name="dsend")
            drecv = dram.tile([N_CORES, FPC, TPB], BF16, name="drecv")

            identity = consts.tile([128, 128], BF16)
            make_identity(nc, identity)

            w_sb = consts.tile([128, 8, 3 * FPC], BF16)  # [d_chunk, dc, f]
            nc.sync.dma_start(
                out=w_sb,
                in_=wqkvt.ap().rearrange("(dc p) f -> p dc f", p=128))
            b_sb = consts.tile([FPC, 3], F32)
            nc.sync.dma_start(out=b_sb, in_=bqkv3[:, :])
            wout_sb = consts.tile([128, 8, D], BF16)  # [f_chunk, fc, e]
            nc.scalar.dma_start(
                out=wout_sb,
                in_=woutt.ap().rearrange("(fc p) e -> p fc e", p=128))
            bout_sb = consts.tile([128, D], F32)
            bout_bcast = bass.AP(
                tensor=boutr.ap().tensor,
                offset=boutr.ap().offset,
                ap=[[0, 128], boutr.ap().ap[1]])
            nc.gpsimd.dma_start(out=bout_sb, in_=bout_bcast)

            # warm-up collective: absorbs one-time CC setup + start skew
            nc.gpsimd.collective_compute(
                "AllToAll", AluOp.bypass,
                replica_groups=[list(range(N_CORES))],
                ins=[dsend.opt()], outs=[drecv.opt()])

            xtb_r = xtb.ap().rearrange("(dc p) t -> p dc t", p=128)

            qkvts = {}
            vps = {}

            # ---------- filler thunks (PE work interleaved into kc slots) ----
            def qkv_supertile_thunks(b, st):
                # QKV projection for batch b, tokens [st*512, (st+1)*512)
                if b not in qkvts:
                    qkvts[b] = qkvt_pool.tile([128, 3, S], BF16, tag="qkvt",
                                              name=f"qkvt{b}")
                    vps[b] = vp_pool.tile([128, N_TKC, HPC, 66], BF16,
                                          tag="vp", name=f"vp{b}")
                    nc.vector.memset(vps[b][:, :, :, 64:65], 1.0)
                qkvt_b, vp_b = qkvts[b], vps[b]
                t0 = b * S + st * STT
                xt = xt_pool.tile([128, 8, STT], BF16, tag="xt",
                                  name=f"xt{b}_{st}")

                def load():
                    nc.sync.dma_start(out=xt, in_=xtb_r[:, :, t0:t0 + STT])

                def proj(fc):
                    ps = mm_ps.tile([128, STT], F32, tag="mm",
                                    name=f"qps{b}_{st}_{fc}")
                    for dc in range(8):
                        nc.tensor.matmul(
                            ps,
                            w_sb[:, dc, fc * FPC:(fc + 1) * FPC],
                            xt[:, dc, :],
                            start=(dc == 0), stop=(dc == 7))
                    nc.vector.tensor_scalar_add(
                        qkvt_b[:, fc, st * STT:(st + 1) * STT], ps,
                        b_sb[:, fc:fc + 1])

                def vprep(kc):
                    pst = tr_ps.tile([128, 128], BF16, tag="tr",
                                     name=f"vtr{b}_{kc}")
                    nc.tensor.transpose(
                        pst, qkvt_b[:, 2, kc * TKC:(kc + 1) * TKC], identity)
                    nc.vector.tensor_copy(
                        vp_b[:, kc, :, 0:DK],
                        pst.rearrange("p (h k) -> p h k", h=HPC))

                thunks = [load]
                thunks += [lambda fc=fc: proj(fc) for fc in range(3)]
                thunks += [lambda kc=kc: vprep(kc)
                           for kc in range(st * (STT // TKC),
                                           (st + 1) * (STT // TKC))]
                return thunks

            def pv_tail(b, qc, ops):
                # normalize token-major PV output straight out of PSUM:
                # ops[:, h, tc*65+64] holds the softmax row-sum
                stg = stg_pool.tile([128, QC // 128, HPC, DK], BF16,
                                    tag="stg", name=f"stg{b}_{qc}")
                for h in range(HPC):
                    for tc in range(QC // 128):
                        rcp = rcp_pool.tile([128, 1], F32, tag="rcp",
                                            name=f"rcp{b}_{qc}_{h}_{tc}")
                        nc.vector.reciprocal(
                            rcp, ops[:, h, tc * 65 + DK:tc * 65 + DK + 1])
                        nc.vector.tensor_scalar_mul(
                            stg[:, tc, h, :], ops[:, h, tc * 65:tc * 65 + DK],
                            rcp)
                return stg

            def norm_thunks(b, qc, stg):
                # feature-major transpose-back + send for q-chunk (b, qc)
                sstg = sstg_pool.tile([128, QC], BF16, tag="sstg",
                                      name=f"sstg{b}_{qc}")

                def sendtr(r):
                    pst = tr_ps.tile([128, 128], BF16, tag="tr",
                                     name=f"str{b}_{qc}_{r}")
                    nc.tensor.transpose(pst, stg[:, r, :, :], identity)
                    nc.vector.tensor_copy(sstg[:, r * 128:(r + 1) * 128], pst)

                def ship(j):
                    d = qc * (QC // TPB) + j
                    nc.sync.dma_start(
                        out=sends[b][d],
                        in_=sstg[:, j * TPB:(j + 1) * TPB])

                thunks = [lambda r=r: sendtr(r) for r in range(QC // 128)]
                thunks += [lambda j=j: ship(j) for j in range(QC // TPB)]
                return thunks

            def outproj_thunks(b):
                osb = osb_pool.tile([128, 8, TPB], BF16, tag="osb",
                                    name=f"osb{b}")

                def load():
                    nc.sync.dma_start(
                        out=osb, in_=recvs[b].rearrange("c p t -> p c t"))

                def chunk(tt, ec):
                    yp = mm_ps.tile([128, 512], F32, tag="mm",
                                    name=f"yp{b}_{tt}_{ec}")
                    for fc in range(8):
                        nc.tensor.matmul(
                            yp,
                            osb[:, fc, tt * 128:(tt + 1) * 128],
                            wout_sb[:, fc, ec * 512:(ec + 1) * 512],
                            start=(fc == 0), stop=(fc == 7))
                    yt = yt_pool.tile([128, 512], F32, tag="yt",
                                      name=f"yt{b}_{tt}_{ec}")
                    nc.vector.tensor_add(
                        yt, yp, bout_sb[:, ec * 512:(ec + 1) * 512])
                    nc.sync.dma_start(
                        out=y[b * TPB + tt * 128:b * TPB + (tt + 1) * 128,
                              ec * 512:(ec + 1) * 512],
                        in_=yt)

                return [load] + [lambda tt=tt, ec=ec: chunk(tt, ec)
                                 for tt in range(TPB // 128)
                                 for ec in range(D // 512)]

            # ---------- main per-iteration emission ----------
            def emit_pv_mms(ops, ppcomb, vp_p, kc):
                # PV with pcomb stationary: O[tq, dk+1] token-major, N=65.
                # 8 chains (h x tc) share 2 banks; exactly one start=True per
                # bank marks the whole zero-region pending (HW has_written
                # semantics), every other first-touch overwrites, later MMs
                # accumulate.
                for h in range(HPC):
                    for tc in range(QC // 128):
                        nc.tensor.matmul(
                            ops[:, h, tc * 65:tc * 65 + DK + 1],
                            ppcomb[:, h, kc, tc * 128:(tc + 1) * 128],
                            vp_p[:, kc, h, 0:DK + 1],
                            start=(kc == 0 and tc == 0),
                            stop=(kc == N_TKC - 1),
                            skip_group_check=True)

            def emit_iter(cur, pv_st, fillers):
                """cur=(b,qc) scores+exp; pv_st=(b,qc,pcomb) PV chains
                interleaved per kc; fillers: list of thunks to spread."""
                b, qc = cur
                qkvt_b = qkvts[b]
                q0 = qc * QC
                pcomb = pcomb_pool.tile([128, HPC, N_TKC, QC], BF16,
                                        tag="pc", name=f"pc{b}_{qc}")
                ops = None
                if pv_st is not None:
                    pb, pqc, ppcomb = pv_st
                    vp_p = vps[pb]
                    ops = o_ps.tile([128, HPC, QC], F32, tag="op",
                                    name=f"op{pb}_{pqc}")
                fq = list(fillers)
                fi = 0
                for kc in range(N_TKC):
                    sp = s_ps.tile([128, HPC, QC], F32, tag="sp",
                                   name=f"sp{b}_{qc}_{kc}")
                    for h in range(HPC):
                        kt = qkvt_b[h * DK:(h + 1) * DK, 1,
                                    kc * TKC:(kc + 1) * TKC]
                        qt = qkvt_b[h * DK:(h + 1) * DK, 0, q0:q0 + QC]
                        nc.tensor.matmul(
                            sp[:, h, :], kt, qt,
                            start=True, stop=True,
                            tile_position=(h * DK, 0))
                    nc.scalar.activation(
                        pcomb[:, :, kc, :], sp, ActFn.Exp, scale=1.0 / 8.0)
                    if pv_st is not None:
                        emit_pv_mms(ops, ppcomb, vp_p, kc)
                    # spread filler thunks proportionally across kc slots
                    while fi < len(fq) and fi * N_TKC <= (kc + 1) * len(fq) - N_TKC:
                        fq[fi]()
                        fi += 1
                # normalize PV output, freeing o_ps for the next iteration
                stg = None
                if pv_st is not None:
                    pb, pqc, _ = pv_st
                    stg = pv_tail(pb, pqc, ops)
                # leftover fillers
                while fi < len(fq):
                    fq[fi]()
                    fi += 1
                return pcomb, stg

            # ---------- pipeline ----------
            for st in range(4):
                for t in qkv_supertile_thunks(0, st):
                    t()

            # iteration stream: (b, qc) for all batches
            iters = [(b, qc) for b in range(B) for qc in range(NQC)]
            pv_st = None      # (b, qc, pcomb) awaiting PV
            nm_st = None      # (b, qc, stg) awaiting transpose-back + send
            for idx, (b, qc) in enumerate(iters):
                fillers = []
                if nm_st is not None:
                    nb, nqc, nstg = nm_st
                    fillers += norm_thunks(nb, nqc, nstg)
                    if nqc == 1 and nb >= 1:
                        fillers += outproj_thunks(nb - 1)
                if b + 1 < B:
                    fillers += qkv_supertile_thunks(b + 1, qc)
                pcomb, stg = emit_iter((b, qc), pv_st, fillers)
                if pv_st is not None:
                    pb, pqc, _ = pv_st
                    if pqc == NQC - 1:
                        # batch-last chunk: ship eagerly and trigger the
                        # AllToAll as soon as its data exists
                        for t in norm_thunks(pb, pqc, stg):
                            t()
                        nc.gpsimd.collective_compute(
                            "AllToAll", AluOp.bypass,
                            replica_groups=[list(range(N_CORES))],
                            ins=[sends[pb].opt()], outs=[recvs[pb].opt()])
                        nm_st = None
                    else:
                        nm_st = (pb, pqc, stg)
                pv_st = (b, qc, pcomb)

            # ---------- epilogue ----------
            # PV for the last q-chunk (dense; all exps done)
            b, qc = iters[-1]
            ops = o_ps.tile([128, HPC, QC], F32, tag="op", name="op_last")
            for kc in range(N_TKC):
                emit_pv_mms(ops, pv_st[2], vps[b], kc)
            stg_last = pv_tail(b, qc, ops)
            # ship (b,2) then (b,3), final collective + outproj
            nb, nqc, nstg = nm_st
            for t in norm_thunks(nb, nqc, nstg):
                t()
            for t in norm_thunks(b, qc, stg_last):
                t()
            nc.gpsimd.collective_compute(
                "AllToAll", AluOp.bypass,
                replica_groups=[list(range(N_CORES))],
                ins=[sends[b].opt()], outs=[recvs[b].opt()])
            for t in outproj_thunks(B - 1):
                t()

    nc.compile()
    return nc


def _get_nc():
    if "nc" not in _CACHE:
        _CACHE["nc"] = _build()
    return _CACHE["nc"]


def kernel(x, Wqkv, bqkv, Wout, bout):
    x = np.asarray(x, dtype=np.float32)
    Wqkv = np.asarray(Wqkv, dtype=np.float32)
    bqkv = np.asarray(bqkv, dtype=np.float32)
    Wout = np.asarray(Wout, dtype=np.float32)
    bout = np.asarray(bout, dtype=np.float32)

    xtb = np.ascontiguousarray(x.reshape(T, D).T.astype(BF16NP))  # [D, T]
    woutt = np.ascontiguousarray(Wout.T.astype(BF16NP))  # [f, e]
    boutr = bout.reshape(1, D)

    in_maps = []
    for c in range(N_CORES):
        f0 = c * FPC  # first feature row of this core's heads
        rows = np.concatenate([
            Wqkv[f0:f0 + FPC],                  # q rows
            Wqkv[D + f0:D + f0 + FPC],          # k rows
            Wqkv[2 * D + f0:2 * D + f0 + FPC],  # v rows
        ])  # [384, 1024]
        wqkvt = np.ascontiguousarray(rows.T.astype(BF16NP))  # [1024, 384]
        bq = np.concatenate([
            bqkv[f0:f0 + FPC],
            bqkv[D + f0:D + f0 + FPC],
            bqkv[2 * D + f0:2 * D + f0 + FPC],
        ])  # [384]
        bqkv3 = np.ascontiguousarray(bq.reshape(3, FPC).T)  # [128, 3]
        in_maps.append({
            "xtb": xtb,
            "wqkvt": wqkvt,
            "bqkv3": bqkv3,
            "woutt": woutt,
            "boutr": boutr,
        })

    nc = _get_nc()
    trace = os.environ.get("MHA_TRACE") == "1"
    res = run_bass_kernel_spmd(
        nc, in_maps, core_ids=list(range(N_CORES)), trace=trace)
    if trace:
        _CACHE["last_result"] = res

    # y_c[b*256 + r] holds global token b*2048 + c*256 + r
    ys = np.stack([res.results[c]["y"].reshape(B, TPB, D)
                   for c in range(N_CORES)], axis=1)  # [B, core, TPB, D]
    return np.ascontiguousarray(ys.reshape(B, S, D))


# revision 21
# speedup vs baseline: 1.5419x; 1.0206x over previous
"""Multi-head attention forward, tensor-parallel over heads across 8 TRN2 cores.

Problem: B=4, S=2048, D=1024, H=16, DK=64.
  qkv = x @ Wqkv.T + bqkv ; per-head scaled-dot-product attention (no mask);
  out = attn_out @ Wout.T + bout

Sharding: 2 heads per core. Work is software-pipelined at q-chunk (512 token)
granularity: iteration i runs scores+exp(i) on PE+ACT while PV(i-1) and
normalize(i-2) interleave into the PE stream as fillers, along with the QKV
projection supertile for the NEXT batch and the output projection for tokens
received from the PREVIOUS batch's AllToAll. ACT (the exp engine) is the
critical resource and is kept saturated; everything else hides behind it.

Key structural points per core:
  - x is pre-transposed / pre-cast to bf16 on the host ([D, T]), so Q^T/K^T
    come out of the QKV matmul feature-major with no PE transposes on x.
  - V' is token-major with a fused ones-column so P@V also yields softmax
    row-sums (PSUM row 64).
  - scores: S^T[tk, tq] = K^T.T @ Q^T, two heads packed as 64-row PE tiles
    (tile_position) writing one 2-bank PSUM slab; one ACT exp op per k-chunk
    covers both heads (scale=1/sqrt(dk)); QC=512 moving dim.
  - normalize: O^T -> PE transpose -> token-major scale by 1/rowsum -> PE
    transpose back to feature-major, so AllToAll receivers can run the out
    projection directly (recv slabs are the matmul lhsT).
  - A tiny warm-up AllToAll issues at kernel start to absorb the one-time
    collective setup / cross-core start skew while early compute runs.
  - Out-proj token ownership: dest core d owns tokens [b*2048 + d*256, +256)
    for every batch b, so each batch forms a complete 8-way AllToAll that
    overlaps the next batch's attention; the host reassembles the result.
"""
import os
import sys

import numpy as np
import ml_dtypes

sys.path.insert(0, "/opt/trn_rl_repo")

import concourse.bass as bass
import concourse.mybir as mybir
import concourse.tile as tile
from concourse import bacc
from concourse.bass_utils import run_bass_kernel_spmd
from concourse.masks import make_identity

F32 = mybir.dt.float32
BF16 = mybir.dt.bfloat16
BF16NP = ml_dtypes.bfloat16

N_CORES = 8
B, S, D, H = 4, 2048, 1024, 16
DK = D // H
T = B * S  # 8192 flattened tokens
HPC = H // N_CORES  # heads per core = 2
FPC = HPC * DK  # features per core = 128
TPC = T // N_CORES  # tokens per core for out-proj = 1024
TPB = TPC // B  # out-proj tokens per core per batch = 256

QC = 512  # q-chunk (moving dim of scores / PV matmuls)
NQC = S // QC  # 4 q-chunks per batch
STT = 512  # QKV token super-tile
TKC = 128  # k-token chunk (partition dim of S^T tiles)
N_TKC = S // TKC  # 16

AluOp = mybir.AluOpType
ActFn = mybir.ActivationFunctionType

_CACHE = {}


def _build():
    nc = bacc.Bacc("TRN2", target_bir_lowering=False, debug=False,
                   num_devices=N_CORES)

    xtb = nc.dram_tensor("xtb", [D, T], BF16, kind="ExternalInput")
    wqkvt = nc.dram_tensor("wqkvt", [D, 3 * FPC], BF16, kind="ExternalInput")
    bqkv3 = nc.dram_tensor("bqkv3", [FPC, 3], F32, kind="ExternalInput")
    woutt = nc.dram_tensor("woutt", [D, D], BF16, kind="ExternalInput")
    boutr = nc.dram_tensor("boutr", [1, D], F32, kind="ExternalInput")
    y = nc.dram_tensor("y", [TPC, D], F32, kind="ExternalOutput")

    with tile.TileContext(nc) as tc:
        with (
            tc.tile_pool(name="dram", bufs=1, space="DRAM") as dram,
            tc.tile_pool(name="consts", bufs=1) as consts,
            tc.tile_pool(name="qkvt", bufs=2) as qkvt_pool,
            tc.tile_pool(name="vp", bufs=3) as vp_pool,
            tc.tile_pool(name="xt", bufs=2) as xt_pool,
            tc.tile_pool(name="pcomb", bufs=2) as pcomb_pool,
            tc.tile_pool(name="stg", bufs=2) as stg_pool,
            tc.tile_pool(name="sstg", bufs=2) as sstg_pool,
            tc.tile_pool(name="osb", bufs=2) as osb_pool,
            tc.tile_pool(name="yt", bufs=2) as yt_pool,
            tc.tile_pool(name="rcp", bufs=4) as rcp_pool,
            tc.tile_pool(name="s_ps", bufs=2, space="PSUM") as s_ps,
            tc.tile_pool(name="o_ps", bufs=1, space="PSUM") as o_ps,
            tc.tile_pool(name="mm_ps", bufs=1, space="PSUM") as mm_ps,
            tc.tile_pool(name="tr_ps", bufs=1, space="PSUM") as tr_ps,
        ):
            sends = [dram.tile([N_CORES, FPC, TPB], BF16, name=f"send{b}")
                     for b in range(B)]
            recvs = [dram.tile([N_CORES, FPC, TPB], BF16, name=f"recv{b}")
                     for b in range(B)]
            dsend = dram.tile([N_CORES, FPC, TPB], BF16, name="dsend")
            drecv = dram.tile([N_CORES, FPC, TPB], BF16, name="drecv")

            identity = consts.tile([128, 128], BF16)
            make_identity(nc, identity)

            w_sb = consts.tile([128, 8, 3 * FPC], BF16)  # [d_chunk, dc, f]
            nc.scalar.dma_start(
                out=w_sb,
                in_=wqkvt.ap().rearrange("(dc p) f -> p dc f", p=128))
            b_sb = consts.tile([FPC, 3], F32)
            nc.scalar.dma_start(out=b_sb, in_=bqkv3[:, :])
            wout_sb = consts.tile([128, 8, D], BF16)  # [f_chunk, fc, e]
            nc.scalar.dma_start(
                out=wout_sb,
                in_=woutt.ap().rearrange("(fc p) e -> p fc e", p=128))
            bout_sb = consts.tile([128, D], F32)
            bout_bcast = bass.AP(
                tensor=boutr.ap().tensor,
                offset=boutr.ap().offset,
                ap=[[0, 128], boutr.ap().ap[1]])
            nc.gpsimd.dma_start(out=bout_sb, in_=bout_bcast)

            # warm-up collective: absorbs one-time CC setup + start skew
            nc.gpsimd.collective_compute(
                "AllToAll", AluOp.bypass,
                replica_groups=[list(range(N_CORES))],
                ins=[dsend.opt()], outs=[drecv.opt()])

            xtb_r = xtb.ap().rearrange("(dc p) t -> p dc t", p=128)

            qkvts = {}
            vps = {}

            # ---------- filler thunks (PE work interleaved into kc slots) ----
            def qkv_supertile_thunks(b, st):
                # QKV projection for batch b, tokens [st*512, (st+1)*512)
                if b not in qkvts:
                    qkvts[b] = qkvt_pool.tile([128, 3, S], BF16, tag="qkvt",
                                              name=f"qkvt{b}")
                    vps[b] = vp_pool.tile([128, N_TKC, HPC, 66], BF16,
                                          tag="vp", name=f"vp{b}")
                    nc.vector.memset(vps[b][:, :, :, 64:65], 1.0)
                qkvt_b, vp_b = qkvts[b], vps[b]
                t0 = b * S + st * STT
                xt = xt_pool.tile([128, 8, STT], BF16, tag="xt",
                                  name=f"xt{b}_{st}")

                def load():
                    nc.sync.dma_start(out=xt, in_=xtb_r[:, :, t0:t0 + STT])

                def proj(fc):
                    ps = mm_ps.tile([128, STT], F32, tag="mm",
                                    name=f"qps{b}_{st}_{fc}")
                    for dc in range(8):
                        nc.tensor.matmul(
                            ps,
                            w_sb[:, dc, fc * FPC:(fc + 1) * FPC],
                            xt[:, dc, :],
                            start=(dc == 0), stop=(dc == 7))
                    nc.vector.tensor_scalar_add(
                        qkvt_b[:, fc, st * STT:(st + 1) * STT], ps,
                        b_sb[:, fc:fc + 1])

                def vprep(kc):
                    pst = tr_ps.tile([128, 128], BF16, tag="tr",
                                     name=f"vtr{b}_{kc}")
                    nc.tensor.transpose(
                        pst, qkvt_b[:, 2, kc * TKC:(kc + 1) * TKC], identity)
                    nc.vector.tensor_copy(
                        vp_b[:, kc, :, 0:DK],
                        pst.rearrange("p (h k) -> p h k", h=HPC))

                thunks = [load]
                thunks += [lambda fc=fc: proj(fc) for fc in range(3)]
                thunks += [lambda kc=kc: vprep(kc)
                           for kc in range(st * (STT // TKC),
                                           (st + 1) * (STT // TKC))]
                return thunks

            def pv_tail(b, qc, ops):
                # normalize token-major PV output straight out of PSUM:
                # ops[:, h, tc*65+64] holds the softmax row-sum
                stg = stg_pool.tile([128, QC // 128, HPC, DK], BF16,
                                    tag="stg", name=f"stg{b}_{qc}")
                for h in range(HPC):
                    for tc in range(QC // 128):
                        rcp = rcp_pool.tile([128, 1], F32, tag="rcp",
                                            name=f"rcp{b}_{qc}_{h}_{tc}")
                        nc.vector.reciprocal(
                            rcp, ops[:, h, tc * 65 + DK:tc * 65 + DK + 1])
                        nc.vector.tensor_scalar_mul(
                            stg[:, tc, h, :], ops[:, h, tc * 65:tc * 65 + DK],
                            rcp)
                return stg

            def norm_thunks(b, qc, stg):
                # feature-major transpose-back + send for q-chunk (b, qc)
                sstg = sstg_pool.tile([128, QC], BF16, tag="sstg",
                                      name=f"sstg{b}_{qc}")

                def sendtr(r):
                    pst = tr_ps.tile([128, 128], BF16, tag="tr",
                                     name=f"str{b}_{qc}_{r}")
                    nc.tensor.transpose(pst, stg[:, r, :, :], identity)
                    nc.vector.tensor_copy(sstg[:, r * 128:(r + 1) * 128], pst)

                def ship(j):
                    d = qc * (QC // TPB) + j
                    nc.gpsimd.dma_start(
                        out=sends[b][d],
                        in_=sstg[:, j * TPB:(j + 1) * TPB])

                thunks = [lambda r=r: sendtr(r) for r in range(QC // 128)]
                thunks += [lambda j=j: ship(j) for j in range(QC // TPB)]
                return thunks

            def outproj_thunks(b):
                osb = osb_pool.tile([128, 8, TPB], BF16, tag="osb",
                                    name=f"osb{b}")

                def load():
                    nc.sync.dma_start(
                        out=osb, in_=recvs[b].rearrange("c p t -> p c t"))

                def chunk(tt, ec):
                    yp = mm_ps.tile([128, 512], F32, tag="mm",
                                    name=f"yp{b}_{tt}_{ec}")
                    for fc in range(8):
                        nc.tensor.matmul(
                            yp,
                            osb[:, fc, tt * 128:(tt + 1) * 128],
                            wout_sb[:, fc, ec * 512:(ec + 1) * 512],
                            start=(fc == 0), stop=(fc == 7))
                    yt = yt_pool.tile([128, 512], F32, tag="yt",
                                      name=f"yt{b}_{tt}_{ec}")
                    nc.vector.tensor_add(
                        yt, yp, bout_sb[:, ec * 512:(ec + 1) * 512])
                    nc.sync.dma_start(
                        out=y[b * TPB + tt * 128:b * TPB + (tt + 1) * 128,
                              ec * 512:(ec + 1) * 512],
                        in_=yt)

                return [load] + [lambda tt=tt, ec=ec: chunk(tt, ec)
                                 for tt in range(TPB // 128)
                                 for ec in range(D // 512)]

            # ---------- main per-iteration emission ----------
            def emit_pv_mms(ops, ppcomb, vp_p, kc):
                # PV with pcomb stationary: O[tq, dk+1] token-major, N=65.
                # 8 chains (h x tc) share 2 banks; exactly one start=True per
                # bank marks the whole zero-region pending (HW has_written
                # semantics), every other first-touch overwrites, later MMs
                # accumulate.
                for h in range(HPC):
                    for tc in range(QC // 128):
                        nc.tensor.matmul(
                            ops[:, h, tc * 65:tc * 65 + DK + 1],
                            ppcomb[:, h, kc, tc * 128:(tc + 1) * 128],
                            vp_p[:, kc, h, 0:DK + 1],
                            start=(kc == 0 and tc == 0),
                            stop=(kc == N_TKC - 1),
                            skip_group_check=True)

            def emit_iter(cur, pv_st, fillers):
                """cur=(b,qc) scores+exp; pv_st=(b,qc,pcomb) PV chains
                interleaved per kc; fillers: list of thunks to spread."""
                b, qc = cur
                qkvt_b = qkvts[b]
                q0 = qc * QC
                pcomb = pcomb_pool.tile([128, HPC, N_TKC, QC], BF16,
                                        tag="pc", name=f"pc{b}_{qc}")
                ops = None
                if pv_st is not None:
                    pb, pqc, ppcomb = pv_st
                    vp_p = vps[pb]
                    ops = o_ps.tile([128, HPC, QC], F32, tag="op",
                                    name=f"op{pb}_{pqc}")
                fq = list(fillers)
                fi = 0
                for kc in range(N_TKC):
                    sp = s_ps.tile([128, HPC, QC], F32, tag="sp",
                                   name=f"sp{b}_{qc}_{kc}")
                    for h in range(HPC):
                        kt = qkvt_b[h * DK:(h + 1) * DK, 1,
                                    kc * TKC:(kc + 1) * TKC]
                        qt = qkvt_b[h * DK:(h + 1) * DK, 0, q0:q0 + QC]
                        nc.tensor.matmul(
                            sp[:, h, :], kt, qt,
                            start=True, stop=True,
                            tile_position=(h * DK, 0))
                    nc.scalar.activation(
                        pcomb[:, :, kc, :], sp, ActFn.Exp, scale=1.0 / 8.0)
                    if pv_st is not None:
                        emit_pv_mms(ops, ppcomb, vp_p, kc)
                    # spread filler thunks proportionally across kc slots
                    while fi < len(fq) and fi * N_TKC <= (kc + 1) * len(fq) - N_TKC:
                        fq[fi]()
                        fi += 1
                # normalize PV output, freeing o_ps for the next iteration
                stg = None
                if pv_st is not None:
                    pb, pqc, _ = pv_st
                    stg = pv_tail(pb, pqc, ops)
                # leftover fillers
                while fi < len(fq):
                    fq[fi]()
                    fi += 1
                return pcomb, stg

            # ---------- pipeline ----------
            for st in range(4):
                for t in qkv_supertile_thunks(0, st):
                    t()

            # iteration stream: (b, qc) for all batches
            iters = [(b, qc) for b in range(B) for qc in range(NQC)]
            pv_st = None      # (b, qc, pcomb) awaiting PV
            nm_st = None      # (b, qc, stg) awaiting transpose-back + send
            for idx, (b, qc) in enumerate(iters):
                fillers = []
                if nm_st is not None:
                    nb, nqc, nstg = nm_st
                    fillers += norm_thunks(nb, nqc, nstg)
                    if nqc == 1 and nb >= 1:
                        fillers += outproj_thunks(nb - 1)
                if b + 1 < B:
                    fillers += qkv_supertile_thunks(b + 1, qc)
                pcomb, stg = emit_iter((b, qc), pv_st, fillers)
                if pv_st is not None:
                    pb, pqc, _ = pv_st
                    if pqc == NQC - 1:
                        # batch-last chunk: ship eagerly and trigger the
                        # AllToAll as soon as its data exists
                        for t in norm_thunks(pb, pqc, stg):
                            t()
                        nc.gpsimd.collective_compute(
                            "AllToAll", AluOp.bypass,
                            replica_groups=[list(range(N_CORES))],
                            ins=[sends[pb].opt()], outs=[recvs[pb].opt()])
                        nm_st = None
                    else:
                        nm_st = (pb, pqc, stg)
                pv_st = (b, qc, pcomb)

            # ---------- epilogue ----------
            # PV for the last q-chunk (dense; all exps done)
            b, qc = iters[-1]
            ops = o_ps.tile([128, HPC, QC], F32, tag="op", name="op_last")
            for kc in range(N_TKC):
                emit_pv_mms(ops, pv_st[2], vps[b], kc)
            stg_last = pv_tail(b, qc, ops)
            # ship (b,2) then (b,3), final collective + outproj
            nb, nqc, nstg = nm_st
            for t in norm_thunks(nb, nqc, nstg):
                t()
            for t in norm_thunks(b, qc, stg_last):
                t()
            nc.gpsimd.collective_compute(
                "AllToAll", AluOp.bypass,
                replica_groups=[list(range(N_CORES))],
                ins=[sends[b].opt()], outs=[recvs[b].opt()])
            # keep the PE HAM-warm across the final collective so the last
            # out-projection runs at full clock
            for i in range(70):
                wk = tr_ps.tile([128, 128], BF16, tag="tr", name=f"wk{i}")
                nc.tensor.transpose(wk, identity, identity)
            for t in outproj_thunks(B - 1):
                t()

    nc.compile()
    return nc


def _get_nc():
    if "nc" not in _CACHE:
        _CACHE["nc"] = _build()
    return _CACHE["nc"]


def kernel(x, Wqkv, bqkv, Wout, bout):
    x = np.asarray(x, dtype=np.float32)
    Wqkv = np.asarray(Wqkv, dtype=np.float32)
    bqkv = np.asarray(bqkv, dtype=np.float32)
    Wout = np.asarray(Wout, dtype=np.float32)
    bout = np.asarray(bout, dtype=np.float32)

    xtb = np.ascontiguousarray(x.reshape(T, D).T.astype(BF16NP))  # [D, T]
    woutt = np.ascontiguousarray(Wout.T.astype(BF16NP))  # [f, e]
    boutr = bout.reshape(1, D)

    in_maps = []
    for c in range(N_CORES):
        f0 = c * FPC  # first feature row of this core's heads
        rows = np.concatenate([
            Wqkv[f0:f0 + FPC],                  # q rows
            Wqkv[D + f0:D + f0 + FPC],          # k rows
            Wqkv[2 * D + f0:2 * D + f0 + FPC],  # v rows
        ])  # [384, 1024]
        wqkvt = np.ascontiguousarray(rows.T.astype(BF16NP))  # [1024, 384]
        bq = np.concatenate([
            bqkv[f0:f0 + FPC],
            bqkv[D + f0:D + f0 + FPC],
            bqkv[2 * D + f0:2 * D + f0 + FPC],
        ])  # [384]
        bqkv3 = np.ascontiguousarray(bq.reshape(3, FPC).T)  # [128, 3]
        in_maps.append({
            "xtb": xtb,
            "wqkvt": wqkvt,
            "bqkv3": bqkv3,
            "woutt": woutt,
            "boutr": boutr,
        })

    nc = _get_nc()
    trace = os.environ.get("MHA_TRACE") == "1"
    res = run_bass_kernel_spmd(
        nc, in_maps, core_ids=list(range(N_CORES)), trace=trace)
    if trace:
        _CACHE["last_result"] = res

    # y_c[b*256 + r] holds global token b*2048 + c*256 + r
    ys = np.stack([res.results[c]["y"].reshape(B, TPB, D)
                   for c in range(N_CORES)], axis=1)  # [B, core, TPB, D]
    return np.ascontiguousarray(ys.reshape(B, S, D))


# revision 27
# speedup vs baseline: 1.5719x; 1.0195x over previous
"""Multi-head attention forward, tensor-parallel over heads across 8 TRN2 cores.

Problem: B=4, S=2048, D=1024, H=16, DK=64.
  qkv = x @ Wqkv.T + bqkv ; per-head scaled-dot-product attention (no mask);
  out = attn_out @ Wout.T + bout

Sharding: 2 heads per core. Work is software-pipelined at q-chunk (512 token)
granularity: iteration i runs scores+exp(i) on PE+ACT while PV(i-1) and
normalize(i-2) interleave into the PE stream as fillers, along with the QKV
projection supertile for the NEXT batch and the output projection for tokens
received from the PREVIOUS batch's AllToAll. ACT (the exp engine) is the
critical resource and is kept saturated; everything else hides behind it.

Key structural points per core:
  - x is pre-transposed / pre-cast to bf16 on the host ([D, T]), so Q^T/K^T
    come out of the QKV matmul feature-major with no PE transposes on x.
  - V' is token-major with a fused ones-column so P@V also yields softmax
    row-sums (PSUM row 64).
  - scores: S^T[tk, tq] = K^T.T @ Q^T, two heads packed as 64-row PE tiles
    (tile_position) writing one 2-bank PSUM slab; one ACT exp op per k-chunk
    covers both heads (scale=1/sqrt(dk)); QC=512 moving dim.
  - normalize: O^T -> PE transpose -> token-major scale by 1/rowsum -> PE
    transpose back to feature-major, so AllToAll receivers can run the out
    projection directly (recv slabs are the matmul lhsT).
  - A tiny warm-up AllToAll issues at kernel start to absorb the one-time
    collective setup / cross-core start skew while early compute runs.
  - Out-proj token ownership: dest core d owns tokens [b*2048 + d*256, +256)
    for every batch b, so each batch forms a complete 8-way AllToAll that
    overlaps the next batch's attention; the host reassembles the result.
"""
import os
import sys

import numpy as np
import ml_dtypes

sys.path.insert(0, "/opt/trn_rl_repo")

import concourse.bass as bass
import concourse.mybir as mybir
import concourse.tile as tile
from concourse import bacc
from concourse.bass_utils import run_bass_kernel_spmd
from concourse.masks import make_identity

F32 = mybir.dt.float32
BF16 = mybir.dt.bfloat16
BF16NP = ml_dtypes.bfloat16

N_CORES = 8
B, S, D, H = 4, 2048, 1024, 16
DK = D // H
T = B * S  # 8192 flattened tokens
HPC = H // N_CORES  # heads per core = 2
FPC = HPC * DK  # features per core = 128
TPC = T // N_CORES  # tokens per core for out-proj = 1024
TPB = TPC // B  # out-proj tokens per core per batch = 256

QC = 512  # q-chunk (moving dim of scores / PV matmuls)
NQC = S // QC  # 4 q-chunks per batch
STT = 512  # QKV token super-tile
TKC = 128  # k-token chunk (partition dim of S^T tiles)
N_TKC = S // TKC  # 16

AluOp = mybir.AluOpType
ActFn = mybir.ActivationFunctionType

_CACHE = {}


def _build():
    nc = bacc.Bacc("TRN2", target_bir_lowering=False, debug=False,
                   num_devices=N_CORES)

    # xtb[st, p, dc*512+t] = x[st*512+t, dc*128+p] (host-tiled x^T, bf16)
    xtb = nc.dram_tensor("xtb", [T // STT, 128, 8 * STT], BF16,
                         kind="ExternalInput")
    # wqkvt[p, dc*384+f] = Wqkv_rows^T[dc*128+p, f]
    wqkvt = nc.dram_tensor("wqkvt", [128, 8 * 3 * FPC], BF16,
                           kind="ExternalInput")
    bqkv3 = nc.dram_tensor("bqkv3", [FPC, 3], F32, kind="ExternalInput")
    # woutt[p, fc*1024+e] = Wout^T[fc*128+p, e]
    woutt = nc.dram_tensor("woutt", [128, 8 * D], BF16, kind="ExternalInput")
    boutr = nc.dram_tensor("boutr", [1, D], F32, kind="ExternalInput")
    y = nc.dram_tensor("y", [TPC, D], F32, kind="ExternalOutput")

    with tile.TileContext(nc) as tc:
        with (
            tc.tile_pool(name="dram", bufs=1, space="DRAM") as dram,
            tc.tile_pool(name="consts", bufs=1) as consts,
            tc.tile_pool(name="qkvt", bufs=2) as qkvt_pool,
            tc.tile_pool(name="vp", bufs=3) as vp_pool,
            tc.tile_pool(name="xt", bufs=2) as xt_pool,
            tc.tile_pool(name="pcomb", bufs=2) as pcomb_pool,
            tc.tile_pool(name="stg", bufs=2) as stg_pool,
            tc.tile_pool(name="sstg", bufs=2) as sstg_pool,
            tc.tile_pool(name="osb", bufs=2) as osb_pool,
            tc.tile_pool(name="yt", bufs=2) as yt_pool,
            tc.tile_pool(name="rcp", bufs=4) as rcp_pool,
            tc.tile_pool(name="s_ps", bufs=2, space="PSUM") as s_ps,
            tc.tile_pool(name="o_ps", bufs=1, space="PSUM") as o_ps,
            tc.tile_pool(name="mm_ps", bufs=1, space="PSUM") as mm_ps,
            tc.tile_pool(name="tr_ps", bufs=1, space="PSUM") as tr_ps,
        ):
            sends = [dram.tile([N_CORES, FPC, TPB], BF16, name=f"send{b}")
                     for b in range(B)]
            recvs = [dram.tile([N_CORES, FPC, TPB], BF16, name=f"recv{b}")
                     for b in range(B)]
            dsend = dram.tile([N_CORES, FPC, TPB], BF16, name="dsend")
            drecv = dram.tile([N_CORES, FPC, TPB], BF16, name="drecv")

            identity = consts.tile([128, 128], BF16)
            make_identity(nc, identity)

            w_sb = consts.tile([128, 8, 3 * FPC], BF16)  # [d_chunk, dc, f]
            nc.scalar.dma_start(out=w_sb, in_=wqkvt[:, :])
            b_sb = consts.tile([FPC, 3], F32)
            nc.scalar.dma_start(out=b_sb, in_=bqkv3[:, :])
            wout_sb = consts.tile([128, 8, D], BF16)  # [f_chunk, fc, e]
            nc.scalar.dma_start(out=wout_sb, in_=woutt[:, :])
            bout_sb = consts.tile([128, D], F32)
            bout_bcast = bass.AP(
                tensor=boutr.ap().tensor,
                offset=boutr.ap().offset,
                ap=[[0, 128], boutr.ap().ap[1]])
            nc.gpsimd.dma_start(out=bout_sb, in_=bout_bcast)

            # warm-up collective: absorbs one-time CC setup + start skew
            nc.gpsimd.collective_compute(
                "AllToAll", AluOp.bypass,
                replica_groups=[list(range(N_CORES))],
                ins=[dsend.opt()], outs=[drecv.opt()])

            qkvts = {}
            vps = {}

            # ---------- filler thunks (PE work interleaved into kc slots) ----
            def qkv_supertile_thunks(b, st):
                # QKV projection for batch b, tokens [st*512, (st+1)*512)
                if b not in qkvts:
                    qkvts[b] = qkvt_pool.tile([128, 3, S], BF16, tag="qkvt",
                                              name=f"qkvt{b}")
                    vps[b] = vp_pool.tile([128, N_TKC, HPC, 66], BF16,
                                          tag="vp", name=f"vp{b}")
                    nc.vector.memset(vps[b][:, :, :, 64:65], 1.0)
                qkvt_b, vp_b = qkvts[b], vps[b]
                sti = b * (S // STT) + st
                xt = xt_pool.tile([128, 8, STT], BF16, tag="xt",
                                  name=f"xt{b}_{st}")

                def load():
                    nc.sync.dma_start(
                        out=xt,
                        in_=xtb[sti].rearrange("p (dc t) -> p dc t", dc=8))

                def proj(fc):
                    ps = mm_ps.tile([128, STT], F32, tag="mm",
                                    name=f"qps{b}_{st}_{fc}")
                    for dc in range(8):
                        nc.tensor.matmul(
                            ps,
                            w_sb[:, dc, fc * FPC:(fc + 1) * FPC],
                            xt[:, dc, :],
                            start=(dc == 0), stop=(dc == 7))
                    nc.vector.tensor_scalar_add(
                        qkvt_b[:, fc, st * STT:(st + 1) * STT], ps,
                        b_sb[:, fc:fc + 1])

                def vprep(kc):
                    pst = tr_ps.tile([128, 128], BF16, tag="tr",
                                     name=f"vtr{b}_{kc}")
                    nc.tensor.transpose(
                        pst, qkvt_b[:, 2, kc * TKC:(kc + 1) * TKC], identity)
                    nc.vector.tensor_copy(
                        vp_b[:, kc, :, 0:DK],
                        pst.rearrange("p (h k) -> p h k", h=HPC))

                thunks = [load]
                thunks += [lambda fc=fc: proj(fc) for fc in range(3)]
                thunks += [lambda kc=kc: vprep(kc)
                           for kc in range(st * (STT // TKC),
                                           (st + 1) * (STT // TKC))]
                return thunks

            def pv_tail(b, qc, ops):
                # normalize token-major PV output straight out of PSUM:
                # ops[:, h, tc*65+64] holds the softmax row-sum
                stg = stg_pool.tile([128, QC // 128, HPC, DK], BF16,
                                    tag="stg", name=f"stg{b}_{qc}")
                for h in range(HPC):
                    for tc in range(QC // 128):
                        rcp = rcp_pool.tile([128, 1], F32, tag="rcp",
                                            name=f"rcp{b}_{qc}_{h}_{tc}")
                        nc.vector.reciprocal(
                            rcp, ops[:, h, tc * 65 + DK:tc * 65 + DK + 1])
                        nc.vector.tensor_scalar_mul(
                            stg[:, tc, h, :], ops[:, h, tc * 65:tc * 65 + DK],
                            rcp)
                return stg

            def norm_thunks(b, qc, stg):
                # feature-major transpose-back + send for q-chunk (b, qc)
                sstg = sstg_pool.tile([128, QC], BF16, tag="sstg",
                                      name=f"sstg{b}_{qc}")

                def sendtr(r):
                    pst = tr_ps.tile([128, 128], BF16, tag="tr",
                                     name=f"str{b}_{qc}_{r}")
                    nc.tensor.transpose(pst, stg[:, r, :, :], identity)
                    nc.vector.tensor_copy(sstg[:, r * 128:(r + 1) * 128], pst)

                def ship(j):
                    d = qc * (QC // TPB) + j
                    nc.gpsimd.dma_start(
                        out=sends[b][d],
                        in_=sstg[:, j * TPB:(j + 1) * TPB])

                thunks = [lambda r=r: sendtr(r) for r in range(QC // 128)]
                thunks += [lambda j=j: ship(j) for j in range(QC // TPB)]
                return thunks

            def outproj_thunks(b):
                osb = osb_pool.tile([128, 8, TPB], BF16, tag="osb",
                                    name=f"osb{b}")

                def load():
                    nc.sync.dma_start(
                        out=osb, in_=recvs[b].rearrange("c p t -> p c t"))

                def chunk(tt, ec):
                    yp = mm_ps.tile([128, 512], F32, tag="mm",
                                    name=f"yp{b}_{tt}_{ec}")
                    for fc in range(8):
                        nc.tensor.matmul(
                            yp,
                            osb[:, fc, tt * 128:(tt + 1) * 128],
                            wout_sb[:, fc, ec * 512:(ec + 1) * 512],
                            start=(fc == 0), stop=(fc == 7))
                    yt = yt_pool.tile([128, 512], F32, tag="yt",
                                      name=f"yt{b}_{tt}_{ec}")
                    nc.vector.tensor_add(
                        yt, yp, bout_sb[:, ec * 512:(ec + 1) * 512])
                    nc.sync.dma_start(
                        out=y[b * TPB + tt * 128:b * TPB + (tt + 1) * 128,
                              ec * 512:(ec + 1) * 512],
                        in_=yt)

                return [load] + [lambda tt=tt, ec=ec: chunk(tt, ec)
                                 for tt in range(TPB // 128)
                                 for ec in range(D // 512)]

            # ---------- main per-iteration emission ----------
            def emit_pv_mms(ops, ppcomb, vp_p, kc):
                # PV with pcomb stationary: O[tq, dk+1] token-major, N=65.
                # 8 chains (h x tc) share 2 banks; exactly one start=True per
                # bank marks the whole zero-region pending (HW has_written
                # semantics), every other first-touch overwrites, later MMs
                # accumulate.
                for h in range(HPC):
                    for tc in range(QC // 128):
                        nc.tensor.matmul(
                            ops[:, h, tc * 65:tc * 65 + DK + 1],
                            ppcomb[:, h, kc, tc * 128:(tc + 1) * 128],
                            vp_p[:, kc, h, 0:DK + 1],
                            start=(kc == 0 and tc == 0),
                            stop=(kc == N_TKC - 1),
                            skip_group_check=True)

            def emit_iter(cur, pv_st, fillers):
                """cur=(b,qc) scores+exp; pv_st=(b,qc,pcomb) PV chains
                interleaved per kc; fillers: list of thunks to spread."""
                b, qc = cur
                qkvt_b = qkvts[b]
                q0 = qc * QC
                pcomb = pcomb_pool.tile([128, HPC, N_TKC, QC], BF16,
                                        tag="pc", name=f"pc{b}_{qc}")
                ops = None
                if pv_st is not None:
                    pb, pqc, ppcomb = pv_st
                    vp_p = vps[pb]
                    ops = o_ps.tile([128, HPC, QC], F32, tag="op",
                                    name=f"op{pb}_{pqc}")
                fq = list(fillers)
                fi = 0
                for kc in range(N_TKC):
                    sp = s_ps.tile([128, HPC, QC], F32, tag="sp",
                                   name=f"sp{b}_{qc}_{kc}")
                    for h in range(HPC):
                        kt = qkvt_b[h * DK:(h + 1) * DK, 1,
                                    kc * TKC:(kc + 1) * TKC]
                        qt = qkvt_b[h * DK:(h + 1) * DK, 0, q0:q0 + QC]
                        nc.tensor.matmul(
                            sp[:, h, :], kt, qt,
                            start=True, stop=True,
                            tile_position=(h * DK, 0))
                    nc.scalar.activation(
                        pcomb[:, :, kc, :], sp, ActFn.Exp, scale=1.0 / 8.0)
                    if pv_st is not None:
                        emit_pv_mms(ops, ppcomb, vp_p, kc)
                    # spread filler thunks proportionally across kc slots
                    while fi < len(fq) and fi * N_TKC <= (kc + 1) * len(fq) - N_TKC:
                        fq[fi]()
                        fi += 1
                # normalize PV output, freeing o_ps for the next iteration
                stg = None
                if pv_st is not None:
                    pb, pqc, _ = pv_st
                    stg = pv_tail(pb, pqc, ops)
                # leftover fillers
                while fi < len(fq):
                    fq[fi]()
                    fi += 1
                return pcomb, stg

            # ---------- pipeline ----------
            for st in range(4):
                for t in qkv_supertile_thunks(0, st):
                    t()

            # iteration stream: (b, qc) for all batches
            iters = [(b, qc) for b in range(B) for qc in range(NQC)]
            pv_st = None      # (b, qc, pcomb) awaiting PV
            nm_st = None      # (b, qc, stg) awaiting transpose-back + send
            for idx, (b, qc) in enumerate(iters):
                fillers = []
                if nm_st is not None:
                    nb, nqc, nstg = nm_st
                    fillers += norm_thunks(nb, nqc, nstg)
                    if nqc == 1 and nb >= 1:
                        fillers += outproj_thunks(nb - 1)
                if b + 1 < B:
                    fillers += qkv_supertile_thunks(b + 1, qc)
                pcomb, stg = emit_iter((b, qc), pv_st, fillers)
                if pv_st is not None:
                    pb, pqc, _ = pv_st
                    if pqc == NQC - 1:
                        # batch-last chunk: ship eagerly and trigger the
                        # AllToAll as soon as its data exists
                        for t in norm_thunks(pb, pqc, stg):
                            t()
                        nc.gpsimd.collective_compute(
                            "AllToAll", AluOp.bypass,
                            replica_groups=[list(range(N_CORES))],
                            ins=[sends[pb].opt()], outs=[recvs[pb].opt()])
                        nm_st = None
                    else:
                        nm_st = (pb, pqc, stg)
                pv_st = (b, qc, pcomb)

            # ---------- epilogue ----------
            # PV for the last q-chunk (dense; all exps done)
            b, qc = iters[-1]
            ops = o_ps.tile([128, HPC, QC], F32, tag="op", name="op_last")
            for kc in range(N_TKC):
                emit_pv_mms(ops, pv_st[2], vps[b], kc)
            stg_last = pv_tail(b, qc, ops)
            # ship (b,2) then (b,3), final collective + outproj
            nb, nqc, nstg = nm_st
            for t in norm_thunks(nb, nqc, nstg):
                t()
            for t in norm_thunks(b, qc, stg_last):
                t()
            nc.gpsimd.collective_compute(
                "AllToAll", AluOp.bypass,
                replica_groups=[list(range(N_CORES))],
                ins=[sends[b].opt()], outs=[recvs[b].opt()])
            # keep the PE HAM-warm across the final collective so the last
            # out-projection runs at full clock
            for i in range(70):
                wk = tr_ps.tile([128, 128], BF16, tag="tr", name=f"wk{i}")
                nc.tensor.transpose(wk, identity, identity)
            for t in outproj_thunks(B - 1):
                t()

    nc.compile()
    return nc


def _get_nc():
    if "nc" not in _CACHE:
        _CACHE["nc"] = _build()
    return _CACHE["nc"]


def kernel(x, Wqkv, bqkv, Wout, bout):
    x = np.asarray(x, dtype=np.float32)
    Wqkv = np.asarray(Wqkv, dtype=np.float32)
    bqkv = np.asarray(bqkv, dtype=np.float32)
    Wout = np.asarray(Wout, dtype=np.float32)
    bout = np.asarray(bout, dtype=np.float32)

    # tiled x^T: xtb[st, p, dc, t] = x[st*512+t, dc*128+p], 8KB DMA lines
    xtb = np.ascontiguousarray(
        x.reshape(T // STT, STT, 8, 128).transpose(0, 3, 2, 1)
        .astype(BF16NP).reshape(T // STT, 128, 8 * STT))
    # tiled Wout^T: woutt[p, fc, e] = Wout.T[fc*128+p, e]
    woutt = np.ascontiguousarray(
        Wout.T.reshape(8, 128, D).transpose(1, 0, 2)
        .astype(BF16NP).reshape(128, 8 * D))
    boutr = bout.reshape(1, D)

    in_maps = []
    for c in range(N_CORES):
        f0 = c * FPC  # first feature row of this core's heads
        rows = np.concatenate([
            Wqkv[f0:f0 + FPC],                  # q rows
            Wqkv[D + f0:D + f0 + FPC],          # k rows
            Wqkv[2 * D + f0:2 * D + f0 + FPC],  # v rows
        ])  # [384, 1024]
        # tiled: wqkvt[p, dc, f] = rows.T[dc*128+p, f]
        wqkvt = np.ascontiguousarray(
            rows.T.reshape(8, 128, 3 * FPC).transpose(1, 0, 2)
            .astype(BF16NP).reshape(128, 8 * 3 * FPC))
        bq = np.concatenate([
            bqkv[f0:f0 + FPC],
            bqkv[D + f0:D + f0 + FPC],
            bqkv[2 * D + f0:2 * D + f0 + FPC],
        ])  # [384]
        bqkv3 = np.ascontiguousarray(bq.reshape(3, FPC).T)  # [128, 3]
        in_maps.append({
            "xtb": xtb,
            "wqkvt": wqkvt,
            "bqkv3": bqkv3,
            "woutt": woutt,
            "boutr": boutr,
        })

    nc = _get_nc()
    trace = os.environ.get("MHA_TRACE") == "1"
    res = run_bass_kernel_spmd(
        nc, in_maps, core_ids=list(range(N_CORES)), trace=trace)
    if trace:
        _CACHE["last_result"] = res

    # y_c[b*256 + r] holds global token b*2048 + c*256 + r
    ys = np.stack([res.results[c]["y"].reshape(B, TPB, D)
                   for c in range(N_CORES)], axis=1)  # [B, core, TPB, D]
    return np.ascontiguousarray(ys.reshape(B, S, D))
